# revision 10
# baseline (speedup 1.0000x reference)
"""GAT (2-layer, PyG-style GATConv) on 8 Trainium2 NeuronCores.

Strategy (dst-sharded, per sharding hint):
- Nodes sharded by dst across 8 cores (12500 each). Edges partitioned by dst
  core; segment-softmax + weighted aggregation happen locally per dst shard.
- Per-edge source rows (h[src], a_src[src]) are fetched with the custom SWDGE
  dma_gather instruction (68B payload rows at 256B stride), 4 table chunks of
  <=25001 rows each to fit int16 indices, round-robined over 4 SWDGE queues.
- Per dst-node tile of 128 (degree-sorted, rectangular per-chunk slot grids),
  softmax + weighted reduction run as wide DVE/ACT ops over [128, K, 17].
- 3 SPMD launches: transform (x@W1 + scores) / layer-1 aggregation + layer-2
  table build / layer-2 aggregation + classifier + log_softmax.
"""

import numpy as np

import concourse.ap_utils as ap_utils
import concourse.bacc as bacc
import concourse.bass as bass
import concourse.mybir as mybir
from concourse.bass import round_up_to_multiple
from concourse.bass_utils import run_bass_kernel_spmd
from concourse.masks import make_identity
from concourse.tile import TileContext

P = 128
NCORES = 8
N = 100000
F_IN = 512
HID = 16
C_OUT = 32
NEG_SLOPE = 0.2
W = HID + 1            # gathered row payload: h (16) + a_src (1)
ROW = 64               # table row stride in fp32 (256B, dma_gather requirement)
CHUNK = 25000          # real rows per index chunk
CHROWS = CHUNK + 1     # +1 dummy row per chunk
NCHUNK = 4
SH = N // NCORES       # real nodes per core
T_TILES = (SH + P - 1) // P
SHP = T_TILES * P      # padded shard size (12544)
NTAB = NCHUNK * CHROWS  # table rows (100004)
NEG_BIG = -1.0e30
MAX_IDX_PER_GATHER = 8192

FP = mybir.dt.float32
I16 = mybir.dt.int16


def _my_dma_gather(gp, out_ap, in_ap, idxs_ap, num_idxs, elem_size,
                   elem_step, queue_num):
    """BassGpSimd.dma_gather (non-transpose, DRAM source) without the
    256B-elem_size restriction; the row stride (elem_step) must still be a
    multiple of 256B."""
    assert idxs_ap.dtype == I16
    assert in_ap.dtype == out_ap.dtype
    assert in_ap.space == bass.MemorySpace.DRAM
    assert idxs_ap.space == bass.MemorySpace.SBUF
    assert out_ap.space == bass.MemorySpace.SBUF
    assert ap_utils.ap_is_contiguous(out_ap.ap[1:])
    assert ap_utils.ap_is_contiguous(idxs_ap.ap[1:])
    assert in_ap.ap[-1][1] == out_ap.ap[-1][1] == elem_size
    assert out_ap.ap[0][1] * out_ap.ap[1][1] == round_up_to_multiple(num_idxs, 128)
    assert in_ap.ap[0][0] == elem_step
    stride_bytes = elem_step * mybir.dt.size(in_ap.dtype)
    assert stride_bytes % 256 == 0 and stride_bytes // 256 < 256
    _in_ap = gp.lower_ap_dma(in_ap, for_custom_bir_dma=True)
    _idxs_ap = gp.lower_ap(idxs_ap)
    _out_ap = gp.lower_ap(out_ap)
    return gp.add_instruction(
        mybir.InstDMAGatherAnt(
            name=gp.bass.get_next_instruction_name(),
            ins=[*_in_ap, _idxs_ap, gp.lower_val_access(gp.to_reg(num_idxs))],
            outs=[_out_ap],
            transpose=False,
            num_idxs=num_idxs,
            elem_size=elem_size,
            stride_bytes_256=stride_bytes // 256,
            gen_mode=0,
            single_packet=False,
            queue_num=queue_num,
        )
    )


# ---------------------------------------------------------------------------
# Host-side preprocessing
# ---------------------------------------------------------------------------

def _table_row_of(pos):
    """Map a logical position 0..N-1 to its padded table row (dummies at the
    end of each chunk)."""
    return (pos // CHUNK) * CHROWS + (pos % CHUNK)


def _edge_lists(edge_index):
    """Per-core edge lists (with self loops) and degree-sorted node order."""
    src = np.asarray(edge_index[0], dtype=np.int64)
    dst = np.asarray(edge_index[1], dtype=np.int64)
    core = (dst // SH).astype(np.int32)
    lists, orders = [], []
    for c in range(NCORES):
        m = core == c
        s_c = src[m].astype(np.int64)
        d_loc = (dst[m] - c * SH).astype(np.int64)
        own = np.arange(SH, dtype=np.int64)
        s_all = np.concatenate([s_c, own + c * SH])
        d_all = np.concatenate([d_loc, own])
        deg = np.bincount(d_all, minlength=SH)
        order = np.argsort(-deg, kind="stable").astype(np.int64)
        lists.append((s_all, d_all))
        orders.append(order)
    return lists, orders


def _schedule(lists, orders, src_pos_map):
    """Per-(tile, chunk) slot schedule shared across cores, plus per-core
    slot->source-position arrays. src_pos_map maps original src id to its
    table position (None = identity). Chunk of an edge = position // CHUNK.

    srcpos[c][slot] holds the source POSITION (in the table's logical
    numbering) or -1 for padding, slots enumerated tile-major, then chunk,
    then slot row j, then partition p.
    """
    chs, poss = [], []
    counts = []
    for c in range(NCORES):
        s_all, d_all = lists[c]
        pos = s_all if src_pos_map is None else src_pos_map[s_all]
        ch = (pos // CHUNK).astype(np.int64)
        cnt = np.zeros((SH, NCHUNK), np.int32)
        np.add.at(cnt, (d_all, ch), 1)
        counts.append(cnt)
        chs.append(ch)
        poss.append(pos)

    K = np.zeros((T_TILES, NCHUNK), np.int32)
    for c in range(NCORES):
        cnt_sorted = counts[c][orders[c]]
        pad = np.zeros((SHP - SH, NCHUNK), np.int32)
        cs = np.concatenate([cnt_sorted, pad]).reshape(T_TILES, P, NCHUNK)
        K = np.maximum(K, cs.max(axis=1))
    K = np.maximum(K, 1)
    assert int(K.max()) * P <= MAX_IDX_PER_GATHER

    slab_off = np.zeros((T_TILES, NCHUNK), np.int64)
    acc = 0
    for t in range(T_TILES):
        for cc in range(NCHUNK):
            slab_off[t, cc] = acc
            acc += int(K[t, cc]) * P

    srcpos = []
    for c in range(NCORES):
        s_all, d_all = lists[c]
        ch, pos = chs[c], poss[c]
        order = orders[c]
        gridpos_of_node = np.full(SH, -1, np.int64)
        gridpos_of_node[order] = np.arange(SH)
        gp_e = gridpos_of_node[d_all]
        t_e = gp_e // P
        p_e = gp_e % P
        bucket = gp_e * NCHUNK + ch
        bo = np.argsort(bucket, kind="stable")
        bsort = bucket[bo]
        rank = np.arange(len(bsort)) - np.searchsorted(bsort, bsort, side="left")
        rank_e = np.empty_like(rank)
        rank_e[bo] = rank
        total = int(K.sum()) * P
        arr = np.full(total, -1, np.int64)
        slotpos = slab_off[t_e, ch] + rank_e * P + p_e
        arr[slotpos] = pos
        srcpos.append(arr)
    return K, srcpos


def _wrap_idx(local_idx):
    """Wrap an int16 index list [M] (M % 128 == 0) into the SWDGE layout
    [128, M//16]: idx i at partition i%16, col i//16, replicated x8."""
    M = local_idx.shape[0]
    w = local_idx.reshape(M // 16, 16).T.astype(np.int16)  # [16, M//16]
    return np.tile(w, (8, 1))


def _build_idx_tensor(srcpos_arr):
    """Translate slot source positions to chunk-local padded-table indices and
    wrap. Padding slots (-1) use the local dummy index CHUNK. The chunk of a
    slot is fixed by the slab structure, so the local index is pos % CHUNK."""
    a = srcpos_arr
    out = np.empty(a.shape[0], np.int16)
    pad = a < 0
    out[~pad] = (a[~pad] % CHUNK).astype(np.int16)
    out[pad] = CHUNK
    return _wrap_idx(out)


# ---------------------------------------------------------------------------
# Device programs
# ---------------------------------------------------------------------------

def _build_transform():
    """Launch 1: per core, h = xT_shard.T @ W1, a_s = h@att_src, a_d = h@att_dst.
    Inputs : xt [F_IN, SH] fp32 (pre-transposed shard), w1 [F_IN//P, P, HID],
             att [128, 2*HID] (att_src tiled | att_dst tiled)
    Outputs: tabs [SHP, ROW] (cols 0:17 = h|a_s), ad [SHP, 1]
    """
    nc = bacc.Bacc("TRN2", target_bir_lowering=False, debug=False,
                   num_devices=NCORES)
    xt = nc.dram_tensor("xt", [F_IN, SH], FP, kind="ExternalInput").ap()
    w1 = nc.dram_tensor("w1", [F_IN // P, P, HID], FP, kind="ExternalInput").ap()
    att = nc.dram_tensor("att", [P, 2 * HID], FP, kind="ExternalInput").ap()
    tabs = nc.dram_tensor("tabs", [SHP, ROW], FP, kind="ExternalOutput").ap()
    ad = nc.dram_tensor("ad", [SHP, 1], FP, kind="ExternalOutput").ap()
    KC = F_IN // P
    with TileContext(nc) as tc:
        with tc.tile_pool(name="cst", bufs=1) as cst, \
             tc.tile_pool(name="xk", bufs=3) as xk, \
             tc.tile_pool(name="hp", bufs=3) as hp, \
             tc.tile_pool(name="ps", bufs=2, space="PSUM") as ps:
            w1t = cst.tile([P, KC * HID], FP)
            nc.sync.dma_start(out=w1t[:].rearrange("p (k h) -> p k h", k=KC),
                              in_=w1[:].rearrange("k p h -> p k h"))
            attt = cst.tile([P, 2 * HID], FP)
            nc.sync.dma_start(out=attt[:], in_=att[:])
            for t in range(T_TILES):
                m0 = t * P
                mn = min(P, SH - m0)
                xtile = xk.tile([P, KC * P], FP)
                nc.sync.dma_start(
                    out=xtile[:].rearrange("p (k m) -> p k m", k=KC)[:, :, 0:mn],
                    in_=xt[:, m0:m0 + mn].rearrange("(k p) m -> p k m", p=P))
                psum = ps.tile([P, HID], FP, space="PSUM")
                for k in range(KC):
                    nc.tensor.matmul(
                        psum[:mn, :],
                        lhsT=xtile[:, k * P:k * P + mn],
                        rhs=w1t[:, k * HID:(k + 1) * HID],
                        start=(k == 0), stop=(k == KC - 1))
                row = hp.tile([P, W], FP)
                adcol = hp.tile([P, 1], FP)
                if mn < P:
                    nc.vector.memset(row[:], 0.0)
                nc.scalar.copy(row[:mn, 0:HID], psum[:mn, 0:HID])
                scr1 = hp.tile([P, HID], FP, tag="scratch")
                nc.vector.tensor_tensor(out=scr1[:], in0=row[:, 0:HID],
                                        in1=attt[:, 0:HID],
                                        op=mybir.AluOpType.mult)
                nc.vector.tensor_reduce(row[:, HID:HID + 1], scr1[:],
                                        axis=mybir.AxisListType.X,
                                        op=mybir.AluOpType.add)
                scr2 = hp.tile([P, HID], FP, tag="scratch2")
                nc.vector.tensor_tensor(out=scr2[:], in0=row[:, 0:HID],
                                        in1=attt[:, HID:2 * HID],
                                        op=mybir.AluOpType.mult)
                nc.vector.tensor_reduce(adcol[:], scr2[:],
                                        axis=mybir.AxisListType.X,
                                        op=mybir.AluOpType.add)
                nc.sync.dma_start(out=tabs[m0:m0 + P, 0:W], in_=row[:])
                nc.sync.dma_start(out=ad[m0:m0 + P, :], in_=adcol[:])
    nc.compile()
    return nc


def _build_aggregate(K, layer):
    """Launches 2 & 3: grid gather + segment softmax + weighted aggregation.

    layer == 1:
      out per tile: h' = relu(num/den + b1); table2 row [h'|a_s2]; ad2.
      Inputs: tab [NTAB, ROW], idx [128, TOTW], adg [SHP, 1],
              vecs [128, 4*HID] = (b1 | u2 | v2 | unused) tiled
      Outputs: tabs [SHP, ROW], ad [SHP, 1]
    layer == 2:
      out per tile: log_softmax(num/den @ W2 + b2)
      Inputs: tab, idx, adg, vecs [128, 2*C_OUT] = (b2 | unused), w2 [HID, C_OUT]
      Outputs: y [SHP, C_OUT]
    """
    nc = bacc.Bacc("TRN2", target_bir_lowering=False, debug=False,
                   num_devices=NCORES, num_swdge_queues=4)
    tot_slots = int(K.sum()) * P
    TOTW = tot_slots // 16
    tab = nc.dram_tensor("tab", [NTAB, ROW], FP, kind="ExternalInput").ap()
    idx = nc.dram_tensor("idx", [P, TOTW], I16, kind="ExternalInput").ap()
    adg = nc.dram_tensor("adg", [SHP, 1], FP, kind="ExternalInput").ap()
    if layer == 1:
        vecs = nc.dram_tensor("vecs", [P, 4 * HID], FP, kind="ExternalInput").ap()
        tabs = nc.dram_tensor("tabs", [SHP, ROW], FP, kind="ExternalOutput").ap()
        ad = nc.dram_tensor("ad", [SHP, 1], FP, kind="ExternalOutput").ap()
    else:
        vecs = nc.dram_tensor("vecs", [P, 2 * C_OUT], FP, kind="ExternalInput").ap()
        w2 = nc.dram_tensor("w2", [HID, C_OUT], FP, kind="ExternalInput").ap()
        y = nc.dram_tensor("y", [SHP, C_OUT], FP, kind="ExternalOutput").ap()

    Ktot = K.sum(axis=1)  # slots per node per tile
    qn = [0]

    with TileContext(nc) as tc:
        with tc.tile_pool(name="cst", bufs=1) as cst, \
             tc.tile_pool(name="ix", bufs=3) as ixp, \
             tc.tile_pool(name="gr", bufs=3) as grp, \
             tc.tile_pool(name="sc", bufs=3) as scp, \
             tc.tile_pool(name="ou", bufs=3) as oup, \
             tc.tile_pool(name="ps", bufs=2, space="PSUM") as ps:
            vt = cst.tile([P, vecs.shape[1]], FP)
            nc.sync.dma_start(out=vt[:], in_=vecs[:])
            if layer == 2:
                w2t = cst.tile([HID, C_OUT], FP)
                nc.sync.dma_start(out=w2t[:], in_=w2[:])
                ident = cst.tile([P, P], FP)
                make_identity(nc, ident[:])
            slot_off = 0   # running slot offset (per-partition slots)
            for t in range(T_TILES):
                kt = int(Ktot[t])
                g = grp.tile([P, kt * W], FP, tag="grid")
                idx_t = ixp.tile([P, kt * 8], I16, tag="idx")
                nc.sync.dma_start(
                    out=idx_t[:],
                    in_=idx[:, slot_off * 8:(slot_off + kt) * 8])
                coff = 0
                for cc in range(NCHUNK):
                    kc = int(K[t, cc])
                    ni = kc * P
                    assert ni <= MAX_IDX_PER_GATHER
                    _my_dma_gather(
                        nc.gpsimd,
                        g[:, coff * W:(coff + kc) * W].rearrange(
                            "p (k w) -> p k w", w=W),
                        tab[cc * CHROWS:, 0:W],
                        idx_t[:, coff * 8:(coff + kc) * 8],
                        ni, W, ROW, qn[0] % 4)
                    qn[0] += 1
                    coff += kc
                adcol = scp.tile([P, 1], FP, tag="adc")
                nc.sync.dma_start(out=adcol[:], in_=adg[t * P:(t + 1) * P, :])
                # e = leaky_relu(a_s + a_d) over [P, kt]
                e = scp.tile([P, kt], FP, tag="e")
                pre = scp.tile([P, kt], FP, tag="pre")
                neg = scp.tile([P, kt], FP, tag="neg")
                as_view = g[:].rearrange("p (k w) -> p k w", w=W)[:, :, HID:HID + 1]
                nc.vector.tensor_scalar_add(
                    pre[:], as_view.rearrange("p k w -> p (k w)"), adcol[:])
                nc.vector.tensor_scalar_min(neg[:], pre[:], 0.0)
                nc.vector.tensor_scalar_max(e[:], pre[:], 0.0)
                nc.vector.tensor_scalar(
                    out=neg[:], in0=neg[:], scalar1=NEG_SLOPE, scalar2=None,
                    op0=mybir.AluOpType.mult)
                nc.vector.tensor_tensor(out=e[:], in0=e[:], in1=neg[:],
                                        op=mybir.AluOpType.add)
                m = scp.tile([P, 1], FP, tag="m")
                nc.vector.tensor_reduce(m[:], e[:], axis=mybir.AxisListType.X,
                                        op=mybir.AluOpType.max, negate=True)
                # m now holds -max; w = exp(e - max), den = sum w
                wts = scp.tile([P, kt], FP, tag="w")
                den = scp.tile([P, 1], FP, tag="den")
                nc.scalar.activation(
                    wts[:], e[:], mybir.ActivationFunctionType.Exp,
                    bias=m[:], scale=1.0, accum_out=den[:])
                inv = scp.tile([P, 1], FP, tag="inv")
                nc.vector.reciprocal(inv[:], den[:])
                nc.vector.tensor_scalar_mul(wts[:], wts[:], inv[:])
                # g *= alpha (broadcast over W columns)
                nc.vector.tensor_tensor(
                    out=g[:].rearrange("p (k w) -> p k w", w=W),
                    in0=g[:].rearrange("p (k w) -> p k w", w=W),
                    in1=wts[:].to_broadcast([P, kt, W]),
                    op=mybir.AluOpType.mult)
                num = oup.tile([P, W], FP, tag="num")
                gv = g[:].rearrange("p (k w) -> p w k", w=W)
                nc.vector.tensor_reduce(num[:], gv, axis=mybir.AxisListType.X,
                                        op=mybir.AluOpType.add)
                if layer == 1:
                    row = oup.tile([P, W], FP, tag="row")
                    adout = oup.tile([P, 1], FP, tag="ado")
                    # h' = relu(num + b1)
                    nc.vector.tensor_tensor(
                        out=row[:, 0:HID], in0=num[:, 0:HID],
                        in1=vt[:, 0:HID], op=mybir.AluOpType.add)
                    nc.vector.tensor_scalar_max(row[:, 0:HID], row[:, 0:HID], 0.0)
                    scr1 = oup.tile([P, HID], FP, tag="s1")
                    nc.vector.tensor_tensor(out=scr1[:], in0=row[:, 0:HID],
                                            in1=vt[:, HID:2 * HID],
                                            op=mybir.AluOpType.mult)
                    nc.vector.tensor_reduce(row[:, HID:HID + 1], scr1[:],
                                            axis=mybir.AxisListType.X,
                                            op=mybir.AluOpType.add)
                    scr2 = oup.tile([P, HID], FP, tag="s2")
                    nc.vector.tensor_tensor(out=scr2[:], in0=row[:, 0:HID],
                                            in1=vt[:, 2 * HID:3 * HID],
                                            op=mybir.AluOpType.mult)
                    nc.vector.tensor_reduce(adout[:], scr2[:],
                                            axis=mybir.AxisListType.X,
                                            op=mybir.AluOpType.add)
                    nc.sync.dma_start(out=tabs[t * P:(t + 1) * P, 0:W], in_=row[:])
                    nc.sync.dma_start(out=ad[t * P:(t + 1) * P, :], in_=adout[:])
                else:
                    # out2 = num[:, :16] @ W2 + b2 -> log_softmax
                    pT = ps.tile([HID, P], FP, space="PSUM", tag="pT")
                    nc.tensor.transpose(pT[:], num[:, 0:HID], ident[:])
                    nT = oup.tile([HID, P], FP, tag="nT")
                    nc.scalar.copy(nT[:], pT[:])
                    p2 = ps.tile([P, C_OUT], FP, space="PSUM", tag="p2")
                    nc.tensor.matmul(p2[:], lhsT=nT[:], rhs=w2t[:],
                                     start=True, stop=True)
                    o = oup.tile([P, C_OUT], FP, tag="o")
                    nc.vector.tensor_tensor(out=o[:], in0=p2[:],
                                            in1=vt[:, 0:C_OUT],
                                            op=mybir.AluOpType.add)
                    mx = scp.tile([P, 1], FP, tag="mx")
                    nc.vector.tensor_reduce(mx[:], o[:],
                                            axis=mybir.AxisListType.X,
                                            op=mybir.AluOpType.max, negate=True)
                    ex = oup.tile([P, C_OUT], FP, tag="ex")
                    se = scp.tile([P, 1], FP, tag="se")
                    nc.scalar.activation(ex[:], o[:],
                                         mybir.ActivationFunctionType.Exp,
                                         bias=mx[:], scale=1.0, accum_out=se[:])
                    ls = scp.tile([P, 1], FP, tag="ls")
                    nc.scalar.activation(ls[:], se[:],
                                         mybir.ActivationFunctionType.Ln)
                    ofs = scp.tile([P, 1], FP, tag="ofs")
                    # ofs = mx(-max) - ln(se);  out = o + ofs
                    nc.vector.tensor_tensor(out=ofs[:], in0=mx[:],
                                            in1=ls[:],
                                            op=mybir.AluOpType.subtract)
                    nc.scalar.activation(o[:], o[:],
                                         mybir.ActivationFunctionType.Identity,
                                         bias=ofs[:], scale=1.0)
                    nc.sync.dma_start(out=y[t * P:(t + 1) * P, :], in_=o[:])
                slot_off += kt
    nc.compile()
    return nc


# ---------------------------------------------------------------------------
# Main entry
# ---------------------------------------------------------------------------

LAST_TIMINGS = {}


def kernel(x, edge_index, W1, att_src1, att_dst1, b1, W2, att_src2, att_dst2, b2):
    import time as _time
    x = np.asarray(x, np.float32)
    W1 = np.asarray(W1, np.float32)
    W2 = np.asarray(W2, np.float32)
    att_src1 = np.asarray(att_src1, np.float32)
    att_dst1 = np.asarray(att_dst1, np.float32)
    att_src2 = np.asarray(att_src2, np.float32)
    att_dst2 = np.asarray(att_dst2, np.float32)
    b1 = np.asarray(b1, np.float32)
    b2 = np.asarray(b2, np.float32)

    print("preprocess...", flush=True)
    lists, orders = _edge_lists(edge_index)
    # layer-2 position map: original id -> position in concat-of-sorted-shards
    pos2 = np.empty(N, np.int64)
    for c in range(NCORES):
        pos2[c * SH + orders[c]] = c * SH + np.arange(SH)

    K1, srcpos1 = _schedule(lists, orders, None)
    K2, srcpos2 = _schedule(lists, orders, pos2)
    idx1 = [_build_idx_tensor(srcpos1[c]) for c in range(NCORES)]
    idx2 = [_build_idx_tensor(srcpos2[c]) for c in range(NCORES)]

    # ---- launch 1: transform -------------------------------------------
    print("build1...", flush=True)
    nc1 = _build_transform()
    xT = np.ascontiguousarray(x.T)
    att_t = np.tile(np.concatenate([att_src1, att_dst1])[None, :], (P, 1))
    w1r = np.ascontiguousarray(W1.reshape(F_IN // P, P, HID))
    in1 = [{"xt": np.ascontiguousarray(xT[:, c * SH:(c + 1) * SH]),
            "w1": w1r, "att": att_t.astype(np.float32)}
           for c in range(NCORES)]
    _t = _time.time()
    r1 = run_bass_kernel_spmd(nc1, in1, list(range(NCORES)))
    LAST_TIMINGS["launch1"] = _time.time() - _t
    print("launch1 done", flush=True)
    tab_rows = np.zeros((NTAB, ROW), np.float32)
    ad1 = np.zeros(N, np.float32)
    for c in range(NCORES):
        hs = r1.results[c]["tabs"][:SH, 0:W]
        ids = c * SH + np.arange(SH)
        tab_rows[_table_row_of(ids), 0:W] = hs
        ad1[ids] = r1.results[c]["ad"][:SH, 0]
    for cc in range(NCHUNK):
        tab_rows[cc * CHROWS + CHUNK, HID] = NEG_BIG  # dummy a_src

    # ---- launch 2: layer-1 aggregation + layer-2 table -----------------
    print("build2...", flush=True)
    nc2 = _build_aggregate(K1, layer=1)
    u2 = W2 @ att_src2
    v2 = W2 @ att_dst2
    vecs1 = np.zeros((P, 4 * HID), np.float32)
    vecs1[:, 0:HID] = b1[None, :]
    vecs1[:, HID:2 * HID] = u2[None, :]
    vecs1[:, 2 * HID:3 * HID] = v2[None, :]
    in2 = []
    for c in range(NCORES):
        adg = np.zeros((SHP, 1), np.float32)
        adg[:SH, 0] = ad1[c * SH + orders[c]]
        in2.append({"tab": tab_rows, "idx": idx1[c], "adg": adg,
                    "vecs": vecs1})
    _t = _time.time()
    r2 = run_bass_kernel_spmd(nc2, in2, list(range(NCORES)))
    LAST_TIMINGS["launch2"] = _time.time() - _t
    print("launch2 done", flush=True)
    tab2 = np.zeros((NTAB, ROW), np.float32)
    ad2 = np.zeros(N, np.float32)
    for c in range(NCORES):
        hs = r2.results[c]["tabs"][:SH, 0:W]
        posn = c * SH + np.arange(SH)
        tab2[_table_row_of(posn), 0:W] = hs
        ad2[posn] = r2.results[c]["ad"][:SH, 0]
    for cc in range(NCHUNK):
        tab2[cc * CHROWS + CHUNK, HID] = NEG_BIG

    # ---- launch 3: layer-2 aggregation + classifier --------------------
    print("build3...", flush=True)
    nc3 = _build_aggregate(K2, layer=2)
    vecs2 = np.zeros((P, 2 * C_OUT), np.float32)
    vecs2[:, 0:C_OUT] = b2[None, :]
    in3 = []
    for c in range(NCORES):
        adg = np.zeros((SHP, 1), np.float32)
        adg[:SH, 0] = ad2[c * SH:(c + 1) * SH]
        in3.append({"tab": tab2, "idx": idx2[c], "adg": adg,
                    "vecs": vecs2, "w2": W2})
    _t = _time.time()
    r3 = run_bass_kernel_spmd(nc3, in3, list(range(NCORES)))
    LAST_TIMINGS["launch3"] = _time.time() - _t
    print("launch3 done", flush=True)

    out = np.zeros((N, C_OUT), np.float32)
    for c in range(NCORES):
        out[c * SH + orders[c]] = r3.results[c]["y"][:SH, :]
    return out


# revision 11
# speedup vs baseline: 3265.3413x; 3265.3413x over previous
"""GAT (2-layer, PyG-style GATConv) on 8 Trainium2 NeuronCores.

Strategy (dst-sharded, per sharding hint):
- Nodes sharded by dst across 8 cores (12500 each). Edges partitioned by dst
  core; segment-softmax + weighted aggregation happen locally per dst shard.
- Per-edge source rows (h[src], a_src[src]) are fetched with the custom SWDGE
  dma_gather instruction (68B payload rows at 256B stride), 4 table chunks of
  <=25001 rows each to fit int16 indices, round-robined over 4 SWDGE queues.
- Per dst-node tile of 128 (degree-sorted, rectangular per-chunk slot grids),
  softmax + weighted reduction run as wide DVE/ACT ops over [128, K, 17].
- 3 SPMD launches: transform (x@W1 + scores) / layer-1 aggregation + layer-2
  table build / layer-2 aggregation + classifier + log_softmax.
"""

import numpy as np

import concourse.ap_utils as ap_utils
import concourse.bacc as bacc
import concourse.bass as bass
import concourse.mybir as mybir
from concourse.bass import round_up_to_multiple
from concourse.bass_utils import run_bass_kernel_spmd
from concourse.masks import make_identity
from concourse.tile import TileContext

P = 128
NCORES = 8
N = 100000
F_IN = 512
HID = 16
C_OUT = 32
NEG_SLOPE = 0.2
W = HID + 1            # gathered row payload: h (16) + a_src (1)
ROW = 64               # table row stride in fp32 (256B, dma_gather requirement)
CHUNK = 25000          # real rows per index chunk
CHROWS = CHUNK + 1     # +1 dummy row per chunk
NCHUNK = 4
SH = N // NCORES       # real nodes per core
T_TILES = (SH + P - 1) // P
SHP = T_TILES * P      # padded shard size (12544)
NTAB = NCHUNK * CHROWS  # table rows (100004)
NEG_BIG = -1.0e30
MAX_IDX_PER_GATHER = 8192

FP = mybir.dt.float32
I16 = mybir.dt.int16


def _my_dma_gather(gp, out_ap, in_ap, idxs_ap, num_idxs, elem_size,
                   elem_step, queue_num):
    """BassGpSimd.dma_gather (non-transpose, DRAM source) without the
    256B-elem_size restriction; the row stride (elem_step) must still be a
    multiple of 256B."""
    assert idxs_ap.dtype == I16
    assert in_ap.dtype == out_ap.dtype
    assert in_ap.space == bass.MemorySpace.DRAM
    assert idxs_ap.space == bass.MemorySpace.SBUF
    assert out_ap.space == bass.MemorySpace.SBUF
    assert ap_utils.ap_is_contiguous(out_ap.ap[1:])
    assert ap_utils.ap_is_contiguous(idxs_ap.ap[1:])
    assert in_ap.ap[-1][1] == out_ap.ap[-1][1] == elem_size
    assert out_ap.ap[0][1] * out_ap.ap[1][1] == round_up_to_multiple(num_idxs, 128)
    assert in_ap.ap[0][0] == elem_step
    stride_bytes = elem_step * mybir.dt.size(in_ap.dtype)
    assert stride_bytes % 256 == 0 and stride_bytes // 256 < 256
    _in_ap = gp.lower_ap_dma(in_ap, for_custom_bir_dma=True)
    _idxs_ap = gp.lower_ap(idxs_ap)
    _out_ap = gp.lower_ap(out_ap)
    return gp.add_instruction(
        mybir.InstDMAGatherAnt(
            name=gp.bass.get_next_instruction_name(),
            ins=[*_in_ap, _idxs_ap, gp.lower_val_access(gp.to_reg(num_idxs))],
            outs=[_out_ap],
            transpose=False,
            num_idxs=num_idxs,
            elem_size=elem_size,
            stride_bytes_256=stride_bytes // 256,
            gen_mode=0,
            single_packet=False,
            queue_num=queue_num,
        )
    )


# ---------------------------------------------------------------------------
# Host-side preprocessing
# ---------------------------------------------------------------------------

def _table_row_of(pos):
    """Map a logical position 0..N-1 to its padded table row (dummies at the
    end of each chunk)."""
    return (pos // CHUNK) * CHROWS + (pos % CHUNK)


def _edge_lists(edge_index):
    """Per-core edge lists (with self loops) and degree-sorted node order."""
    src = np.asarray(edge_index[0], dtype=np.int64)
    dst = np.asarray(edge_index[1], dtype=np.int64)
    core = (dst // SH).astype(np.int32)
    lists, orders = [], []
    for c in range(NCORES):
        m = core == c
        s_c = src[m].astype(np.int64)
        d_loc = (dst[m] - c * SH).astype(np.int64)
        own = np.arange(SH, dtype=np.int64)
        s_all = np.concatenate([s_c, own + c * SH])
        d_all = np.concatenate([d_loc, own])
        deg = np.bincount(d_all, minlength=SH)
        order = np.argsort(-deg, kind="stable").astype(np.int64)
        lists.append((s_all, d_all))
        orders.append(order)
    return lists, orders


def _schedule(lists, orders, src_pos_map):
    """Per-(tile, chunk) slot schedule shared across cores, plus per-core
    slot->source-position arrays. src_pos_map maps original src id to its
    table position (None = identity). Chunk of an edge = position // CHUNK.

    srcpos[c][slot] holds the source POSITION (in the table's logical
    numbering) or -1 for padding, slots enumerated tile-major, then chunk,
    then slot row j, then partition p.
    """
    chs, poss = [], []
    counts = []
    for c in range(NCORES):
        s_all, d_all = lists[c]
        pos = s_all if src_pos_map is None else src_pos_map[s_all]
        ch = (pos // CHUNK).astype(np.int64)
        cnt = np.zeros((SH, NCHUNK), np.int32)
        np.add.at(cnt, (d_all, ch), 1)
        counts.append(cnt)
        chs.append(ch)
        poss.append(pos)

    K = np.zeros((T_TILES, NCHUNK), np.int32)
    for c in range(NCORES):
        cnt_sorted = counts[c][orders[c]]
        pad = np.zeros((SHP - SH, NCHUNK), np.int32)
        cs = np.concatenate([cnt_sorted, pad]).reshape(T_TILES, P, NCHUNK)
        K = np.maximum(K, cs.max(axis=1))
    K = np.maximum(K, 1)
    assert int(K.max()) * P <= MAX_IDX_PER_GATHER

    slab_off = np.zeros((T_TILES, NCHUNK), np.int64)
    acc = 0
    for t in range(T_TILES):
        for cc in range(NCHUNK):
            slab_off[t, cc] = acc
            acc += int(K[t, cc]) * P

    srcpos = []
    for c in range(NCORES):
        s_all, d_all = lists[c]
        ch, pos = chs[c], poss[c]
        order = orders[c]
        gridpos_of_node = np.full(SH, -1, np.int64)
        gridpos_of_node[order] = np.arange(SH)
        gp_e = gridpos_of_node[d_all]
        t_e = gp_e // P
        p_e = gp_e % P
        bucket = gp_e * NCHUNK + ch
        bo = np.argsort(bucket, kind="stable")
        bsort = bucket[bo]
        rank = np.arange(len(bsort)) - np.searchsorted(bsort, bsort, side="left")
        rank_e = np.empty_like(rank)
        rank_e[bo] = rank
        total = int(K.sum()) * P
        arr = np.full(total, -1, np.int64)
        slotpos = slab_off[t_e, ch] + rank_e * P + p_e
        arr[slotpos] = pos
        srcpos.append(arr)
    return K, srcpos


def _wrap_idx(local_idx):
    """Wrap an int16 index list [M] (M % 128 == 0) into the SWDGE layout
    [128, M//16]: idx i at partition i%16, col i//16, replicated x8."""
    M = local_idx.shape[0]
    w = local_idx.reshape(M // 16, 16).T.astype(np.int16)  # [16, M//16]
    return np.tile(w, (8, 1))


def _build_idx_tensor(srcpos_arr):
    """Translate slot source positions to chunk-local padded-table indices and
    wrap. Padding slots (-1) use the local dummy index CHUNK. The chunk of a
    slot is fixed by the slab structure, so the local index is pos % CHUNK."""
    a = srcpos_arr
    out = np.empty(a.shape[0], np.int16)
    pad = a < 0
    out[~pad] = (a[~pad] % CHUNK).astype(np.int16)
    out[pad] = CHUNK
    return _wrap_idx(out)


# ---------------------------------------------------------------------------
# Device programs
# ---------------------------------------------------------------------------

def _build_transform():
    """Launch 1: per core, h = xT_shard.T @ W1, a_s = h@att_src, a_d = h@att_dst.
    Inputs : xt [F_IN, SH] fp32 (pre-transposed shard), w1 [F_IN//P, P, HID],
             att [128, 2*HID] (att_src tiled | att_dst tiled)
    Outputs: tabs [SHP, ROW] (cols 0:17 = h|a_s), ad [SHP, 1]
    """
    nc = bacc.Bacc("TRN2", target_bir_lowering=False, debug=False,
                   num_devices=NCORES)
    xt = nc.dram_tensor("xt", [F_IN, SH], FP, kind="ExternalInput").ap()
    w1 = nc.dram_tensor("w1", [F_IN // P, P, HID], FP, kind="ExternalInput").ap()
    att = nc.dram_tensor("att", [P, 2 * HID], FP, kind="ExternalInput").ap()
    tabs = nc.dram_tensor("tabs", [SHP, ROW], FP, kind="ExternalOutput").ap()
    ad = nc.dram_tensor("ad", [SHP, 1], FP, kind="ExternalOutput").ap()
    KC = F_IN // P
    with TileContext(nc) as tc:
        with tc.tile_pool(name="cst", bufs=1) as cst, \
             tc.tile_pool(name="xk", bufs=3) as xk, \
             tc.tile_pool(name="hp", bufs=3) as hp, \
             tc.tile_pool(name="ps", bufs=2, space="PSUM") as ps:
            w1t = cst.tile([P, KC * HID], FP)
            nc.sync.dma_start(out=w1t[:].rearrange("p (k h) -> p k h", k=KC),
                              in_=w1[:].rearrange("k p h -> p k h"))
            attt = cst.tile([P, 2 * HID], FP)
            nc.sync.dma_start(out=attt[:], in_=att[:])
            for t in range(T_TILES):
                m0 = t * P
                mn = min(P, SH - m0)
                xtile = xk.tile([P, KC * P], FP)
                nc.sync.dma_start(
                    out=xtile[:].rearrange("p (k m) -> p k m", k=KC)[:, :, 0:mn],
                    in_=xt[:, m0:m0 + mn].rearrange("(k p) m -> p k m", p=P))
                psum = ps.tile([P, HID], FP, space="PSUM")
                for k in range(KC):
                    nc.tensor.matmul(
                        psum[:mn, :],
                        lhsT=xtile[:, k * P:k * P + mn],
                        rhs=w1t[:, k * HID:(k + 1) * HID],
                        start=(k == 0), stop=(k == KC - 1))
                row = hp.tile([P, W], FP)
                adcol = hp.tile([P, 1], FP)
                if mn < P:
                    nc.vector.memset(row[:], 0.0)
                nc.scalar.copy(row[:mn, 0:HID], psum[:mn, 0:HID])
                scr1 = hp.tile([P, HID], FP, tag="scratch")
                nc.vector.tensor_tensor(out=scr1[:], in0=row[:, 0:HID],
                                        in1=attt[:, 0:HID],
                                        op=mybir.AluOpType.mult)
                nc.vector.tensor_reduce(row[:, HID:HID + 1], scr1[:],
                                        axis=mybir.AxisListType.X,
                                        op=mybir.AluOpType.add)
                scr2 = hp.tile([P, HID], FP, tag="scratch2")
                nc.vector.tensor_tensor(out=scr2[:], in0=row[:, 0:HID],
                                        in1=attt[:, HID:2 * HID],
                                        op=mybir.AluOpType.mult)
                nc.vector.tensor_reduce(adcol[:], scr2[:],
                                        axis=mybir.AxisListType.X,
                                        op=mybir.AluOpType.add)
                nc.sync.dma_start(out=tabs[m0:m0 + P, 0:W], in_=row[:])
                nc.sync.dma_start(out=ad[m0:m0 + P, :], in_=adcol[:])
    nc.compile()
    return nc


def _build_aggregate(K, layer):
    """Launches 2 & 3: grid gather + segment softmax + weighted aggregation.

    layer == 1:
      out per tile: h' = relu(num/den + b1); table2 row [h'|a_s2]; ad2.
      Inputs: tab [NTAB, ROW], idx [128, TOTW], adg [SHP, 1],
              vecs [128, 4*HID] = (b1 | u2 | v2 | unused) tiled
      Outputs: tabs [SHP, ROW], ad [SHP, 1]
    layer == 2:
      out per tile: log_softmax(num/den @ W2 + b2)
      Inputs: tab, idx, adg, vecs [128, 2*C_OUT] = (b2 | unused), w2 [HID, C_OUT]
      Outputs: y [SHP, C_OUT]
    """
    nc = bacc.Bacc("TRN2", target_bir_lowering=False, debug=False,
                   num_devices=NCORES, num_swdge_queues=4)
    tot_slots = int(K.sum()) * P
    TOTW = tot_slots // 16
    tab = nc.dram_tensor("tab", [NTAB, ROW], FP, kind="ExternalInput").ap()
    idx = nc.dram_tensor("idx", [P, TOTW], I16, kind="ExternalInput").ap()
    adg = nc.dram_tensor("adg", [SHP, 1], FP, kind="ExternalInput").ap()
    if layer == 1:
        vecs = nc.dram_tensor("vecs", [P, 4 * HID], FP, kind="ExternalInput").ap()
        tabs = nc.dram_tensor("tabs", [SHP, ROW], FP, kind="ExternalOutput").ap()
        ad = nc.dram_tensor("ad", [SHP, 1], FP, kind="ExternalOutput").ap()
    else:
        vecs = nc.dram_tensor("vecs", [P, 2 * C_OUT], FP, kind="ExternalInput").ap()
        w2 = nc.dram_tensor("w2", [HID, C_OUT], FP, kind="ExternalInput").ap()
        y = nc.dram_tensor("y", [SHP, C_OUT], FP, kind="ExternalOutput").ap()

    Ktot = K.sum(axis=1)  # slots per node per tile
    qn = [0]

    with TileContext(nc) as tc:
        with tc.tile_pool(name="cst", bufs=1) as cst, \
             tc.tile_pool(name="ix", bufs=3) as ixp, \
             tc.tile_pool(name="gr", bufs=3) as grp, \
             tc.tile_pool(name="sc", bufs=3) as scp, \
             tc.tile_pool(name="ou", bufs=3) as oup, \
             tc.tile_pool(name="ps", bufs=2, space="PSUM") as ps:
            vt = cst.tile([P, vecs.shape[1]], FP)
            nc.sync.dma_start(out=vt[:], in_=vecs[:])
            if layer == 2:
                w2t = cst.tile([HID, C_OUT], FP)
                nc.sync.dma_start(out=w2t[:], in_=w2[:])
                ident = cst.tile([P, P], FP)
                make_identity(nc, ident[:])
            slot_off = 0   # running slot offset (per-partition slots)
            for t in range(T_TILES):
                kt = int(Ktot[t])
                g = grp.tile([P, kt * W], FP, tag="grid")
                idx_t = ixp.tile([P, kt * 8], I16, tag="idx")
                nc.sync.dma_start(
                    out=idx_t[:],
                    in_=idx[:, slot_off * 8:(slot_off + kt) * 8])
                coff = 0
                for cc in range(NCHUNK):
                    kc = int(K[t, cc])
                    ni = kc * P
                    assert ni <= MAX_IDX_PER_GATHER
                    _my_dma_gather(
                        nc.gpsimd,
                        g[:, coff * W:(coff + kc) * W].rearrange(
                            "p (k w) -> p k w", w=W),
                        tab[cc * CHROWS:, 0:W],
                        idx_t[:, coff * 8:(coff + kc) * 8],
                        ni, W, ROW, qn[0] % 4)
                    qn[0] += 1
                    coff += kc
                adcol = scp.tile([P, 1], FP, tag="adc")
                nc.sync.dma_start(out=adcol[:], in_=adg[t * P:(t + 1) * P, :])
                # e = leaky_relu(a_s + a_d) over [P, kt]
                e = scp.tile([P, kt], FP, tag="e")
                pre = scp.tile([P, kt], FP, tag="pre")
                neg = scp.tile([P, kt], FP, tag="neg")
                as_view = g[:].rearrange("p (k w) -> p k w", w=W)[:, :, HID:HID + 1]
                nc.vector.tensor_scalar_add(
                    pre[:], as_view.rearrange("p k w -> p (k w)"), adcol[:])
                nc.vector.tensor_scalar_min(neg[:], pre[:], 0.0)
                nc.vector.tensor_scalar_max(e[:], pre[:], 0.0)
                nc.vector.tensor_scalar(
                    out=neg[:], in0=neg[:], scalar1=NEG_SLOPE, scalar2=None,
                    op0=mybir.AluOpType.mult)
                nc.vector.tensor_tensor(out=e[:], in0=e[:], in1=neg[:],
                                        op=mybir.AluOpType.add)
                m = scp.tile([P, 1], FP, tag="m")
                nc.vector.tensor_reduce(m[:], e[:], axis=mybir.AxisListType.X,
                                        op=mybir.AluOpType.max, negate=True)
                # m now holds -max; w = exp(e - max), den = sum w
                wts = scp.tile([P, kt], FP, tag="w")
                den = scp.tile([P, 1], FP, tag="den")
                nc.scalar.activation(
                    wts[:], e[:], mybir.ActivationFunctionType.Exp,
                    bias=m[:], scale=1.0, accum_out=den[:])
                inv = scp.tile([P, 1], FP, tag="inv")
                nc.vector.reciprocal(inv[:], den[:])
                nc.vector.tensor_scalar_mul(wts[:], wts[:], inv[:])
                # g *= alpha (broadcast over W columns)
                nc.vector.tensor_tensor(
                    out=g[:].rearrange("p (k w) -> p k w", w=W),
                    in0=g[:].rearrange("p (k w) -> p k w", w=W),
                    in1=wts[:].to_broadcast([P, kt, W]),
                    op=mybir.AluOpType.mult)
                num = oup.tile([P, W], FP, tag="num")
                gv = g[:].rearrange("p (k w) -> p w k", w=W)
                nc.vector.tensor_reduce(num[:], gv, axis=mybir.AxisListType.X,
                                        op=mybir.AluOpType.add)
                if layer == 1:
                    row = oup.tile([P, W], FP, tag="row")
                    adout = oup.tile([P, 1], FP, tag="ado")
                    # h' = relu(num + b1)
                    nc.vector.tensor_tensor(
                        out=row[:, 0:HID], in0=num[:, 0:HID],
                        in1=vt[:, 0:HID], op=mybir.AluOpType.add)
                    nc.vector.tensor_scalar_max(row[:, 0:HID], row[:, 0:HID], 0.0)
                    scr1 = oup.tile([P, HID], FP, tag="s1")
                    nc.vector.tensor_tensor(out=scr1[:], in0=row[:, 0:HID],
                                            in1=vt[:, HID:2 * HID],
                                            op=mybir.AluOpType.mult)
                    nc.vector.tensor_reduce(row[:, HID:HID + 1], scr1[:],
                                            axis=mybir.AxisListType.X,
                                            op=mybir.AluOpType.add)
                    scr2 = oup.tile([P, HID], FP, tag="s2")
                    nc.vector.tensor_tensor(out=scr2[:], in0=row[:, 0:HID],
                                            in1=vt[:, 2 * HID:3 * HID],
                                            op=mybir.AluOpType.mult)
                    nc.vector.tensor_reduce(adout[:], scr2[:],
                                            axis=mybir.AxisListType.X,
                                            op=mybir.AluOpType.add)
                    nc.sync.dma_start(out=tabs[t * P:(t + 1) * P, 0:W], in_=row[:])
                    nc.sync.dma_start(out=ad[t * P:(t + 1) * P, :], in_=adout[:])
                else:
                    # out2 = num[:, :16] @ W2 + b2 -> log_softmax
                    pT = ps.tile([HID, P], FP, space="PSUM", tag="pT")
                    nc.tensor.transpose(pT[:], num[:, 0:HID], ident[:])
                    nT = oup.tile([HID, P], FP, tag="nT")
                    nc.scalar.copy(nT[:], pT[:])
                    p2 = ps.tile([P, C_OUT], FP, space="PSUM", tag="p2")
                    nc.tensor.matmul(p2[:], lhsT=nT[:], rhs=w2t[:],
                                     start=True, stop=True)
                    o = oup.tile([P, C_OUT], FP, tag="o")
                    nc.vector.tensor_tensor(out=o[:], in0=p2[:],
                                            in1=vt[:, 0:C_OUT],
                                            op=mybir.AluOpType.add)
                    mx = scp.tile([P, 1], FP, tag="mx")
                    nc.vector.tensor_reduce(mx[:], o[:],
                                            axis=mybir.AxisListType.X,
                                            op=mybir.AluOpType.max, negate=True)
                    ex = oup.tile([P, C_OUT], FP, tag="ex")
                    se = scp.tile([P, 1], FP, tag="se")
                    nc.scalar.activation(ex[:], o[:],
                                         mybir.ActivationFunctionType.Exp,
                                         bias=mx[:], scale=1.0, accum_out=se[:])
                    ls = scp.tile([P, 1], FP, tag="ls")
                    nc.scalar.activation(ls[:], se[:],
                                         mybir.ActivationFunctionType.Ln)
                    ofs = scp.tile([P, 1], FP, tag="ofs")
                    # ofs = mx(-max) - ln(se);  out = o + ofs
                    nc.vector.tensor_tensor(out=ofs[:], in0=mx[:],
                                            in1=ls[:],
                                            op=mybir.AluOpType.subtract)
                    nc.scalar.activation(o[:], o[:],
                                         mybir.ActivationFunctionType.Identity,
                                         bias=ofs[:], scale=1.0)
                    nc.sync.dma_start(out=y[t * P:(t + 1) * P, :], in_=o[:])
                slot_off += kt
    nc.compile()
    return nc


# ---------------------------------------------------------------------------
# Main entry
# ---------------------------------------------------------------------------

LAST_TIMINGS = {}
LAST_STATS = {}


def _run_retry(nc, in_maps, cores):
    try:
        return run_bass_kernel_spmd(nc, in_maps, cores)
    except Exception:
        # transient accelerator-unrecoverable states heal on retry
        return run_bass_kernel_spmd(nc, in_maps, cores)


def kernel(x, edge_index, W1, att_src1, att_dst1, b1, W2, att_src2, att_dst2, b2):
    import time as _time
    x = np.asarray(x, np.float32)
    W1 = np.asarray(W1, np.float32)
    W2 = np.asarray(W2, np.float32)
    att_src1 = np.asarray(att_src1, np.float32)
    att_dst1 = np.asarray(att_dst1, np.float32)
    att_src2 = np.asarray(att_src2, np.float32)
    att_dst2 = np.asarray(att_dst2, np.float32)
    b1 = np.asarray(b1, np.float32)
    b2 = np.asarray(b2, np.float32)

    print("preprocess...", flush=True)
    lists, orders = _edge_lists(edge_index)
    # layer-2 position map: original id -> position in concat-of-sorted-shards
    pos2 = np.empty(N, np.int64)
    for c in range(NCORES):
        pos2[c * SH + orders[c]] = c * SH + np.arange(SH)

    K1, srcpos1 = _schedule(lists, orders, None)
    K2, srcpos2 = _schedule(lists, orders, pos2)
    LAST_STATS["slots1"] = int(K1.sum()) * P
    LAST_STATS["slots2"] = int(K2.sum()) * P
    idx1 = [_build_idx_tensor(srcpos1[c]) for c in range(NCORES)]
    idx2 = [_build_idx_tensor(srcpos2[c]) for c in range(NCORES)]

    # ---- launch 1: transform -------------------------------------------
    print("build1...", flush=True)
    nc1 = _build_transform()
    xT = np.ascontiguousarray(x.T)
    att_t = np.tile(np.concatenate([att_src1, att_dst1])[None, :], (P, 1))
    w1r = np.ascontiguousarray(W1.reshape(F_IN // P, P, HID))
    in1 = [{"xt": np.ascontiguousarray(xT[:, c * SH:(c + 1) * SH]),
            "w1": w1r, "att": att_t.astype(np.float32)}
           for c in range(NCORES)]
    _t = _time.time()
    r1 = _run_retry(nc1, in1, list(range(NCORES)))
    LAST_TIMINGS["launch1"] = _time.time() - _t
    print("launch1 done", flush=True)
    tab_rows = np.zeros((NTAB, ROW), np.float32)
    ad1 = np.zeros(N, np.float32)
    for c in range(NCORES):
        hs = r1.results[c]["tabs"][:SH, 0:W]
        ids = c * SH + np.arange(SH)
        tab_rows[_table_row_of(ids), 0:W] = hs
        ad1[ids] = r1.results[c]["ad"][:SH, 0]
    for cc in range(NCHUNK):
        tab_rows[cc * CHROWS + CHUNK, HID] = NEG_BIG  # dummy a_src

    # ---- launch 2: layer-1 aggregation + layer-2 table -----------------
    print("build2...", flush=True)
    nc2 = _build_aggregate(K1, layer=1)
    u2 = W2 @ att_src2
    v2 = W2 @ att_dst2
    vecs1 = np.zeros((P, 4 * HID), np.float32)
    vecs1[:, 0:HID] = b1[None, :]
    vecs1[:, HID:2 * HID] = u2[None, :]
    vecs1[:, 2 * HID:3 * HID] = v2[None, :]
    in2 = []
    for c in range(NCORES):
        adg = np.zeros((SHP, 1), np.float32)
        adg[:SH, 0] = ad1[c * SH + orders[c]]
        in2.append({"tab": tab_rows, "idx": idx1[c], "adg": adg,
                    "vecs": vecs1})
    _t = _time.time()
    r2 = _run_retry(nc2, in2, list(range(NCORES)))
    LAST_TIMINGS["launch2"] = _time.time() - _t
    print("launch2 done", flush=True)
    tab2 = np.zeros((NTAB, ROW), np.float32)
    ad2 = np.zeros(N, np.float32)
    for c in range(NCORES):
        hs = r2.results[c]["tabs"][:SH, 0:W]
        posn = c * SH + np.arange(SH)
        tab2[_table_row_of(posn), 0:W] = hs
        ad2[posn] = r2.results[c]["ad"][:SH, 0]
    for cc in range(NCHUNK):
        tab2[cc * CHROWS + CHUNK, HID] = NEG_BIG

    # ---- launch 3: layer-2 aggregation + classifier --------------------
    print("build3...", flush=True)
    nc3 = _build_aggregate(K2, layer=2)
    vecs2 = np.zeros((P, 2 * C_OUT), np.float32)
    vecs2[:, 0:C_OUT] = b2[None, :]
    in3 = []
    for c in range(NCORES):
        adg = np.zeros((SHP, 1), np.float32)
        adg[:SH, 0] = ad2[c * SH:(c + 1) * SH]
        in3.append({"tab": tab2, "idx": idx2[c], "adg": adg,
                    "vecs": vecs2, "w2": W2})
    _t = _time.time()
    r3 = _run_retry(nc3, in3, list(range(NCORES)))
    LAST_TIMINGS["launch3"] = _time.time() - _t
    print("launch3 done", flush=True)

    out = np.zeros((N, C_OUT), np.float32)
    for c in range(NCORES):
        out[c * SH + orders[c]] = r3.results[c]["y"][:SH, :]
    return out


# revision 12
# speedup vs baseline: 3693.2756x; 1.1311x over previous
"""GAT (2-layer, PyG-style GATConv) on 8 Trainium2 NeuronCores.

Strategy (dst-sharded, per sharding hint):
- Nodes sharded by dst across 8 cores (12500 each). Edges partitioned by dst
  core; segment-softmax + weighted aggregation happen locally per dst shard.
- Per-edge source rows (h[src], a_src[src]) are fetched with the custom SWDGE
  dma_gather instruction (68B payload rows at 256B stride), 4 table chunks of
  <=25001 rows each to fit int16 indices, round-robined over 4 SWDGE queues.
- Per dst-node tile of 128 (degree-sorted, rectangular per-chunk slot grids),
  softmax + weighted reduction run as wide DVE/ACT ops over [128, K, 17].
- 3 SPMD launches: transform (x@W1 + scores) / layer-1 aggregation + layer-2
  table build / layer-2 aggregation + classifier + log_softmax.
"""

import numpy as np

import concourse.ap_utils as ap_utils
import concourse.bacc as bacc
import concourse.bass as bass
import concourse.mybir as mybir
from concourse.bass import round_up_to_multiple
from concourse.bass_utils import run_bass_kernel_spmd
from concourse.masks import make_identity
from concourse.tile import TileContext

P = 128
NCORES = 8
N = 100000
F_IN = 512
HID = 16
C_OUT = 32
NEG_SLOPE = 0.2
W = HID + 1            # gathered row payload: h (16) + a_src (1)
ROW = 64               # table row stride in fp32 (256B, dma_gather requirement)
CHUNK = 25000          # real rows per index chunk
CHROWS = CHUNK + 1     # +1 dummy row per chunk
NCHUNK = 4
SH = N // NCORES       # real nodes per core
T_TILES = (SH + P - 1) // P
SHP = T_TILES * P      # padded shard size (12544)
NTAB = NCHUNK * CHROWS  # table rows (100004)
NEG_BIG = -1.0e30
MAX_IDX_PER_GATHER = 8192

FP = mybir.dt.float32
I16 = mybir.dt.int16


def _my_dma_gather(gp, out_ap, in_ap, idxs_ap, num_idxs, elem_size,
                   elem_step, queue_num):
    """BassGpSimd.dma_gather (non-transpose, DRAM source) without the
    256B-elem_size restriction; the row stride (elem_step) must still be a
    multiple of 256B."""
    assert idxs_ap.dtype == I16
    assert in_ap.dtype == out_ap.dtype
    assert in_ap.space == bass.MemorySpace.DRAM
    assert idxs_ap.space == bass.MemorySpace.SBUF
    assert out_ap.space == bass.MemorySpace.SBUF
    assert ap_utils.ap_is_contiguous(out_ap.ap[1:])
    assert ap_utils.ap_is_contiguous(idxs_ap.ap[1:])
    assert in_ap.ap[-1][1] == out_ap.ap[-1][1] == elem_size
    assert out_ap.ap[0][1] * out_ap.ap[1][1] == round_up_to_multiple(num_idxs, 128)
    assert in_ap.ap[0][0] == elem_step
    stride_bytes = elem_step * mybir.dt.size(in_ap.dtype)
    assert stride_bytes % 256 == 0 and stride_bytes // 256 < 256
    _in_ap = gp.lower_ap_dma(in_ap, for_custom_bir_dma=True)
    _idxs_ap = gp.lower_ap(idxs_ap)
    _out_ap = gp.lower_ap(out_ap)
    return gp.add_instruction(
        mybir.InstDMAGatherAnt(
            name=gp.bass.get_next_instruction_name(),
            ins=[*_in_ap, _idxs_ap, gp.lower_val_access(gp.to_reg(num_idxs))],
            outs=[_out_ap],
            transpose=False,
            num_idxs=num_idxs,
            elem_size=elem_size,
            stride_bytes_256=stride_bytes // 256,
            gen_mode=0,
            single_packet=False,
            queue_num=queue_num,
        )
    )


# ---------------------------------------------------------------------------
# Host-side preprocessing
# ---------------------------------------------------------------------------

def _table_row_of(pos):
    """Map a logical position 0..N-1 to its padded table row (dummies at the
    end of each chunk)."""
    return (pos // CHUNK) * CHROWS + (pos % CHUNK)


def _edge_lists(edge_index):
    """Per-core edge lists (with self loops) and degree-sorted node order."""
    src = np.asarray(edge_index[0], dtype=np.int64)
    dst = np.asarray(edge_index[1], dtype=np.int64)
    core = (dst // SH).astype(np.int32)
    lists, orders = [], []
    for c in range(NCORES):
        m = core == c
        s_c = src[m].astype(np.int64)
        d_loc = (dst[m] - c * SH).astype(np.int64)
        own = np.arange(SH, dtype=np.int64)
        s_all = np.concatenate([s_c, own + c * SH])
        d_all = np.concatenate([d_loc, own])
        deg = np.bincount(d_all, minlength=SH)
        order = np.argsort(-deg, kind="stable").astype(np.int64)
        lists.append((s_all, d_all))
        orders.append(order)
    return lists, orders


def _orders_for(lists, src_pos_map):
    """Per-core node order clustering nodes by (degree bucket, per-chunk
    counts) so per-(tile, chunk) maxima sit close to the means."""
    orders = []
    for c in range(NCORES):
        s_all, d_all = lists[c]
        pos = s_all if src_pos_map is None else src_pos_map[s_all]
        ch = pos // CHUNK
        cnt = np.zeros((SH, NCHUNK), np.int32)
        np.add.at(cnt, (d_all, ch), 1)
        tot = cnt.sum(axis=1) // 4
        orders.append(np.lexsort(
            (-cnt[:, 2], -cnt[:, 1], -cnt[:, 0], -tot)).astype(np.int64))
    return orders


def _schedule(lists, orders, src_pos_map):
    """Per-(tile, chunk) slot schedule shared across cores, plus per-core
    slot->source-position arrays. src_pos_map maps original src id to its
    table position (None = identity). Chunk of an edge = position // CHUNK.

    srcpos[c][slot] holds the source POSITION (in the table's logical
    numbering) or -1 for padding, slots enumerated tile-major, then chunk,
    then slot row j, then partition p.
    """
    chs, poss = [], []
    counts = []
    for c in range(NCORES):
        s_all, d_all = lists[c]
        pos = s_all if src_pos_map is None else src_pos_map[s_all]
        ch = (pos // CHUNK).astype(np.int64)
        cnt = np.zeros((SH, NCHUNK), np.int32)
        np.add.at(cnt, (d_all, ch), 1)
        counts.append(cnt)
        chs.append(ch)
        poss.append(pos)

    K = np.zeros((T_TILES, NCHUNK), np.int32)
    for c in range(NCORES):
        cnt_sorted = counts[c][orders[c]]
        pad = np.zeros((SHP - SH, NCHUNK), np.int32)
        cs = np.concatenate([cnt_sorted, pad]).reshape(T_TILES, P, NCHUNK)
        K = np.maximum(K, cs.max(axis=1))
    K = np.maximum(K, 1)
    assert int(K.max()) * P <= MAX_IDX_PER_GATHER

    slab_off = np.zeros((T_TILES, NCHUNK), np.int64)
    acc = 0
    for t in range(T_TILES):
        for cc in range(NCHUNK):
            slab_off[t, cc] = acc
            acc += int(K[t, cc]) * P

    srcpos = []
    for c in range(NCORES):
        s_all, d_all = lists[c]
        ch, pos = chs[c], poss[c]
        order = orders[c]
        gridpos_of_node = np.full(SH, -1, np.int64)
        gridpos_of_node[order] = np.arange(SH)
        gp_e = gridpos_of_node[d_all]
        t_e = gp_e // P
        p_e = gp_e % P
        bucket = gp_e * NCHUNK + ch
        bo = np.argsort(bucket, kind="stable")
        bsort = bucket[bo]
        rank = np.arange(len(bsort)) - np.searchsorted(bsort, bsort, side="left")
        rank_e = np.empty_like(rank)
        rank_e[bo] = rank
        total = int(K.sum()) * P
        arr = np.full(total, -1, np.int64)
        slotpos = slab_off[t_e, ch] + rank_e * P + p_e
        arr[slotpos] = pos
        srcpos.append(arr)
    return K, srcpos


def _wrap_idx(local_idx):
    """Wrap an int16 index list [M] (M % 128 == 0) into the SWDGE layout
    [128, M//16]: idx i at partition i%16, col i//16, replicated x8."""
    M = local_idx.shape[0]
    w = local_idx.reshape(M // 16, 16).T.astype(np.int16)  # [16, M//16]
    return np.tile(w, (8, 1))


def _build_idx_tensor(srcpos_arr):
    """Translate slot source positions to chunk-local padded-table indices and
    wrap. Padding slots (-1) use the local dummy index CHUNK. The chunk of a
    slot is fixed by the slab structure, so the local index is pos % CHUNK."""
    a = srcpos_arr
    out = np.empty(a.shape[0], np.int16)
    pad = a < 0
    out[~pad] = (a[~pad] % CHUNK).astype(np.int16)
    out[pad] = CHUNK
    return _wrap_idx(out)


# ---------------------------------------------------------------------------
# Device programs
# ---------------------------------------------------------------------------

def _build_transform():
    """Launch 1: per core, h = xT_shard.T @ W1, a_s = h@att_src, a_d = h@att_dst.
    Inputs : xt [F_IN, SH] fp32 (pre-transposed shard), w1 [F_IN//P, P, HID],
             att [128, 2*HID] (att_src tiled | att_dst tiled)
    Outputs: tabs [SHP, ROW] (cols 0:17 = h|a_s), ad [SHP, 1]
    """
    nc = bacc.Bacc("TRN2", target_bir_lowering=False, debug=False,
                   num_devices=NCORES)
    xt = nc.dram_tensor("xt", [F_IN, SH], FP, kind="ExternalInput").ap()
    w1 = nc.dram_tensor("w1", [F_IN // P, P, HID], FP, kind="ExternalInput").ap()
    att = nc.dram_tensor("att", [P, 2 * HID], FP, kind="ExternalInput").ap()
    tabs = nc.dram_tensor("tabs", [SHP, ROW], FP, kind="ExternalOutput").ap()
    ad = nc.dram_tensor("ad", [SHP, 1], FP, kind="ExternalOutput").ap()
    KC = F_IN // P
    with TileContext(nc) as tc:
        with tc.tile_pool(name="cst", bufs=1) as cst, \
             tc.tile_pool(name="xk", bufs=3) as xk, \
             tc.tile_pool(name="hp", bufs=3) as hp, \
             tc.tile_pool(name="ps", bufs=2, space="PSUM") as ps:
            w1t = cst.tile([P, KC * HID], FP)
            nc.sync.dma_start(out=w1t[:].rearrange("p (k h) -> p k h", k=KC),
                              in_=w1[:].rearrange("k p h -> p k h"))
            attt = cst.tile([P, 2 * HID], FP)
            nc.sync.dma_start(out=attt[:], in_=att[:])
            for t in range(T_TILES):
                m0 = t * P
                mn = min(P, SH - m0)
                xtile = xk.tile([P, KC * P], FP)
                nc.sync.dma_start(
                    out=xtile[:].rearrange("p (k m) -> p k m", k=KC)[:, :, 0:mn],
                    in_=xt[:, m0:m0 + mn].rearrange("(k p) m -> p k m", p=P))
                psum = ps.tile([P, HID], FP, space="PSUM")
                for k in range(KC):
                    nc.tensor.matmul(
                        psum[:mn, :],
                        lhsT=xtile[:, k * P:k * P + mn],
                        rhs=w1t[:, k * HID:(k + 1) * HID],
                        start=(k == 0), stop=(k == KC - 1))
                row = hp.tile([P, W], FP)
                adcol = hp.tile([P, 1], FP)
                if mn < P:
                    nc.vector.memset(row[:], 0.0)
                nc.scalar.copy(row[:mn, 0:HID], psum[:mn, 0:HID])
                scr1 = hp.tile([P, HID], FP, tag="scratch")
                nc.vector.tensor_tensor(out=scr1[:], in0=row[:, 0:HID],
                                        in1=attt[:, 0:HID],
                                        op=mybir.AluOpType.mult)
                nc.vector.tensor_reduce(row[:, HID:HID + 1], scr1[:],
                                        axis=mybir.AxisListType.X,
                                        op=mybir.AluOpType.add)
                scr2 = hp.tile([P, HID], FP, tag="scratch2")
                nc.vector.tensor_tensor(out=scr2[:], in0=row[:, 0:HID],
                                        in1=attt[:, HID:2 * HID],
                                        op=mybir.AluOpType.mult)
                nc.vector.tensor_reduce(adcol[:], scr2[:],
                                        axis=mybir.AxisListType.X,
                                        op=mybir.AluOpType.add)
                nc.sync.dma_start(out=tabs[m0:m0 + P, 0:W], in_=row[:])
                nc.sync.dma_start(out=ad[m0:m0 + P, :], in_=adcol[:])
    nc.compile()
    return nc


def _build_aggregate(K, layer):
    """Launches 2 & 3: grid gather + segment softmax + weighted aggregation.

    layer == 1:
      out per tile: h' = relu(num/den + b1); table2 row [h'|a_s2]; ad2.
      Inputs: tab [NTAB, ROW], idx [128, TOTW], adg [SHP, 1],
              vecs [128, 4*HID] = (b1 | u2 | v2 | unused) tiled
      Outputs: tabs [SHP, ROW], ad [SHP, 1]
    layer == 2:
      out per tile: log_softmax(num/den @ W2 + b2)
      Inputs: tab, idx, adg, vecs [128, 2*C_OUT] = (b2 | unused), w2 [HID, C_OUT]
      Outputs: y [SHP, C_OUT]
    """
    nc = bacc.Bacc("TRN2", target_bir_lowering=False, debug=False,
                   num_devices=NCORES, num_swdge_queues=4)
    tot_slots = int(K.sum()) * P
    TOTW = tot_slots // 16
    tab = nc.dram_tensor("tab", [NTAB, ROW], FP, kind="ExternalInput").ap()
    idx = nc.dram_tensor("idx", [P, TOTW], I16, kind="ExternalInput").ap()
    adg = nc.dram_tensor("adg", [SHP, 1], FP, kind="ExternalInput").ap()
    if layer == 1:
        vecs = nc.dram_tensor("vecs", [P, 4 * HID], FP, kind="ExternalInput").ap()
        tabs = nc.dram_tensor("tabs", [SHP, ROW], FP, kind="ExternalOutput").ap()
        ad = nc.dram_tensor("ad", [SHP, 1], FP, kind="ExternalOutput").ap()
    else:
        vecs = nc.dram_tensor("vecs", [P, 2 * C_OUT], FP, kind="ExternalInput").ap()
        w2 = nc.dram_tensor("w2", [HID, C_OUT], FP, kind="ExternalInput").ap()
        y = nc.dram_tensor("y", [SHP, C_OUT], FP, kind="ExternalOutput").ap()

    Ktot = K.sum(axis=1)  # slots per node per tile
    qn = [0]

    with TileContext(nc) as tc:
        with tc.tile_pool(name="cst", bufs=1) as cst, \
             tc.tile_pool(name="ix", bufs=3) as ixp, \
             tc.tile_pool(name="gr", bufs=3) as grp, \
             tc.tile_pool(name="sc", bufs=3) as scp, \
             tc.tile_pool(name="ou", bufs=3) as oup, \
             tc.tile_pool(name="ps", bufs=2, space="PSUM") as ps:
            vt = cst.tile([P, vecs.shape[1]], FP)
            nc.sync.dma_start(out=vt[:], in_=vecs[:])
            if layer == 2:
                w2t = cst.tile([HID, C_OUT], FP)
                nc.sync.dma_start(out=w2t[:], in_=w2[:])
                ident = cst.tile([P, P], FP)
                make_identity(nc, ident[:])
            slot_off = 0   # running slot offset (per-partition slots)
            for t in range(T_TILES):
                kt = int(Ktot[t])
                g = grp.tile([P, kt * W], FP, tag="grid")
                idx_t = ixp.tile([P, kt * 8], I16, tag="idx")
                nc.sync.dma_start(
                    out=idx_t[:],
                    in_=idx[:, slot_off * 8:(slot_off + kt) * 8])
                coff = 0
                for cc in range(NCHUNK):
                    kc = int(K[t, cc])
                    ni = kc * P
                    assert ni <= MAX_IDX_PER_GATHER
                    _my_dma_gather(
                        nc.gpsimd,
                        g[:, coff * W:(coff + kc) * W].rearrange(
                            "p (k w) -> p k w", w=W),
                        tab[cc * CHROWS:, 0:W],
                        idx_t[:, coff * 8:(coff + kc) * 8],
                        ni, W, ROW, qn[0] % 4)
                    qn[0] += 1
                    coff += kc
                adcol = scp.tile([P, 1], FP, tag="adc")
                nc.sync.dma_start(out=adcol[:], in_=adg[t * P:(t + 1) * P, :])
                # e = leaky_relu(a_s + a_d) over [P, kt]
                e = scp.tile([P, kt], FP, tag="e")
                pre = scp.tile([P, kt], FP, tag="pre")
                neg = scp.tile([P, kt], FP, tag="neg")
                as_view = g[:].rearrange("p (k w) -> p k w", w=W)[:, :, HID:HID + 1]
                nc.vector.tensor_scalar_add(
                    pre[:], as_view.rearrange("p k w -> p (k w)"), adcol[:])
                nc.vector.tensor_scalar_min(neg[:], pre[:], 0.0)
                nc.vector.tensor_scalar_max(e[:], pre[:], 0.0)
                nc.vector.tensor_scalar(
                    out=neg[:], in0=neg[:], scalar1=NEG_SLOPE, scalar2=None,
                    op0=mybir.AluOpType.mult)
                nc.vector.tensor_tensor(out=e[:], in0=e[:], in1=neg[:],
                                        op=mybir.AluOpType.add)
                m = scp.tile([P, 1], FP, tag="m")
                nc.vector.tensor_reduce(m[:], e[:], axis=mybir.AxisListType.X,
                                        op=mybir.AluOpType.max, negate=True)
                # m now holds -max; w = exp(e - max), den = sum w
                wts = scp.tile([P, kt], FP, tag="w")
                den = scp.tile([P, 1], FP, tag="den")
                nc.scalar.activation(
                    wts[:], e[:], mybir.ActivationFunctionType.Exp,
                    bias=m[:], scale=1.0, accum_out=den[:])
                inv = scp.tile([P, 1], FP, tag="inv")
                nc.vector.reciprocal(inv[:], den[:])
                nc.vector.tensor_scalar_mul(wts[:], wts[:], inv[:])
                # g *= alpha (broadcast over W columns)
                nc.vector.tensor_tensor(
                    out=g[:].rearrange("p (k w) -> p k w", w=W),
                    in0=g[:].rearrange("p (k w) -> p k w", w=W),
                    in1=wts[:].to_broadcast([P, kt, W]),
                    op=mybir.AluOpType.mult)
                num = oup.tile([P, W], FP, tag="num")
                gv = g[:].rearrange("p (k w) -> p w k", w=W)
                nc.vector.tensor_reduce(num[:], gv, axis=mybir.AxisListType.X,
                                        op=mybir.AluOpType.add)
                if layer == 1:
                    row = oup.tile([P, W], FP, tag="row")
                    adout = oup.tile([P, 1], FP, tag="ado")
                    # h' = relu(num + b1)
                    nc.vector.tensor_tensor(
                        out=row[:, 0:HID], in0=num[:, 0:HID],
                        in1=vt[:, 0:HID], op=mybir.AluOpType.add)
                    nc.vector.tensor_scalar_max(row[:, 0:HID], row[:, 0:HID], 0.0)
                    scr1 = oup.tile([P, HID], FP, tag="s1")
                    nc.vector.tensor_tensor(out=scr1[:], in0=row[:, 0:HID],
                                            in1=vt[:, HID:2 * HID],
                                            op=mybir.AluOpType.mult)
                    nc.vector.tensor_reduce(row[:, HID:HID + 1], scr1[:],
                                            axis=mybir.AxisListType.X,
                                            op=mybir.AluOpType.add)
                    scr2 = oup.tile([P, HID], FP, tag="s2")
                    nc.vector.tensor_tensor(out=scr2[:], in0=row[:, 0:HID],
                                            in1=vt[:, 2 * HID:3 * HID],
                                            op=mybir.AluOpType.mult)
                    nc.vector.tensor_reduce(adout[:], scr2[:],
                                            axis=mybir.AxisListType.X,
                                            op=mybir.AluOpType.add)
                    nc.sync.dma_start(out=tabs[t * P:(t + 1) * P, 0:W], in_=row[:])
                    nc.sync.dma_start(out=ad[t * P:(t + 1) * P, :], in_=adout[:])
                else:
                    # out2 = num[:, :16] @ W2 + b2 -> log_softmax
                    pT = ps.tile([HID, P], FP, space="PSUM", tag="pT")
                    nc.tensor.transpose(pT[:], num[:, 0:HID], ident[:])
                    nT = oup.tile([HID, P], FP, tag="nT")
                    nc.scalar.copy(nT[:], pT[:])
                    p2 = ps.tile([P, C_OUT], FP, space="PSUM", tag="p2")
                    nc.tensor.matmul(p2[:], lhsT=nT[:], rhs=w2t[:],
                                     start=True, stop=True)
                    o = oup.tile([P, C_OUT], FP, tag="o")
                    nc.vector.tensor_tensor(out=o[:], in0=p2[:],
                                            in1=vt[:, 0:C_OUT],
                                            op=mybir.AluOpType.add)
                    mx = scp.tile([P, 1], FP, tag="mx")
                    nc.vector.tensor_reduce(mx[:], o[:],
                                            axis=mybir.AxisListType.X,
                                            op=mybir.AluOpType.max, negate=True)
                    ex = oup.tile([P, C_OUT], FP, tag="ex")
                    se = scp.tile([P, 1], FP, tag="se")
                    nc.scalar.activation(ex[:], o[:],
                                         mybir.ActivationFunctionType.Exp,
                                         bias=mx[:], scale=1.0, accum_out=se[:])
                    ls = scp.tile([P, 1], FP, tag="ls")
                    nc.scalar.activation(ls[:], se[:],
                                         mybir.ActivationFunctionType.Ln)
                    ofs = scp.tile([P, 1], FP, tag="ofs")
                    # ofs = mx(-max) - ln(se);  out = o + ofs
                    nc.vector.tensor_tensor(out=ofs[:], in0=mx[:],
                                            in1=ls[:],
                                            op=mybir.AluOpType.subtract)
                    nc.scalar.activation(o[:], o[:],
                                         mybir.ActivationFunctionType.Identity,
                                         bias=ofs[:], scale=1.0)
                    nc.sync.dma_start(out=y[t * P:(t + 1) * P, :], in_=o[:])
                slot_off += kt
    nc.compile()
    return nc


# ---------------------------------------------------------------------------
# Main entry
# ---------------------------------------------------------------------------

LAST_TIMINGS = {}
LAST_STATS = {}


def _run_retry(nc, in_maps, cores):
    try:
        return run_bass_kernel_spmd(nc, in_maps, cores)
    except Exception:
        # transient accelerator-unrecoverable states heal on retry
        return run_bass_kernel_spmd(nc, in_maps, cores)


def kernel(x, edge_index, W1, att_src1, att_dst1, b1, W2, att_src2, att_dst2, b2):
    import time as _time
    x = np.asarray(x, np.float32)
    W1 = np.asarray(W1, np.float32)
    W2 = np.asarray(W2, np.float32)
    att_src1 = np.asarray(att_src1, np.float32)
    att_dst1 = np.asarray(att_dst1, np.float32)
    att_src2 = np.asarray(att_src2, np.float32)
    att_dst2 = np.asarray(att_dst2, np.float32)
    b1 = np.asarray(b1, np.float32)
    b2 = np.asarray(b2, np.float32)

    print("preprocess...", flush=True)
    lists, _ = _edge_lists(edge_index)
    orders1 = _orders_for(lists, None)
    # layer-2 position map: original id -> position in concat-of-sorted-shards
    pos2 = np.empty(N, np.int64)
    for c in range(NCORES):
        pos2[c * SH + orders1[c]] = c * SH + np.arange(SH)
    orders2 = _orders_for(lists, pos2)

    K1, srcpos1 = _schedule(lists, orders1, None)
    K2, srcpos2 = _schedule(lists, orders2, pos2)
    LAST_STATS["slots1"] = int(K1.sum()) * P
    LAST_STATS["slots2"] = int(K2.sum()) * P
    idx1 = [_build_idx_tensor(srcpos1[c]) for c in range(NCORES)]
    idx2 = [_build_idx_tensor(srcpos2[c]) for c in range(NCORES)]

    # ---- launch 1: transform -------------------------------------------
    print("build1...", flush=True)
    nc1 = _build_transform()
    xT = np.ascontiguousarray(x.T)
    att_t = np.tile(np.concatenate([att_src1, att_dst1])[None, :], (P, 1))
    w1r = np.ascontiguousarray(W1.reshape(F_IN // P, P, HID))
    in1 = [{"xt": np.ascontiguousarray(xT[:, c * SH:(c + 1) * SH]),
            "w1": w1r, "att": att_t.astype(np.float32)}
           for c in range(NCORES)]
    _t = _time.time()
    r1 = _run_retry(nc1, in1, list(range(NCORES)))
    LAST_TIMINGS["launch1"] = _time.time() - _t
    print("launch1 done", flush=True)
    tab_rows = np.zeros((NTAB, ROW), np.float32)
    ad1 = np.zeros(N, np.float32)
    for c in range(NCORES):
        hs = r1.results[c]["tabs"][:SH, 0:W]
        ids = c * SH + np.arange(SH)
        tab_rows[_table_row_of(ids), 0:W] = hs
        ad1[ids] = r1.results[c]["ad"][:SH, 0]
    for cc in range(NCHUNK):
        tab_rows[cc * CHROWS + CHUNK, HID] = NEG_BIG  # dummy a_src

    # ---- launch 2: layer-1 aggregation + layer-2 table -----------------
    print("build2...", flush=True)
    nc2 = _build_aggregate(K1, layer=1)
    u2 = W2 @ att_src2
    v2 = W2 @ att_dst2
    vecs1 = np.zeros((P, 4 * HID), np.float32)
    vecs1[:, 0:HID] = b1[None, :]
    vecs1[:, HID:2 * HID] = u2[None, :]
    vecs1[:, 2 * HID:3 * HID] = v2[None, :]
    in2 = []
    for c in range(NCORES):
        adg = np.zeros((SHP, 1), np.float32)
        adg[:SH, 0] = ad1[c * SH + orders1[c]]
        in2.append({"tab": tab_rows, "idx": idx1[c], "adg": adg,
                    "vecs": vecs1})
    _t = _time.time()
    r2 = _run_retry(nc2, in2, list(range(NCORES)))
    LAST_TIMINGS["launch2"] = _time.time() - _t
    print("launch2 done", flush=True)
    tab2 = np.zeros((NTAB, ROW), np.float32)
    ad2 = np.zeros(N, np.float32)
    for c in range(NCORES):
        hs = r2.results[c]["tabs"][:SH, 0:W]
        posn = c * SH + np.arange(SH)
        tab2[_table_row_of(posn), 0:W] = hs
        ad2[posn] = r2.results[c]["ad"][:SH, 0]
    for cc in range(NCHUNK):
        tab2[cc * CHROWS + CHUNK, HID] = NEG_BIG

    # ---- launch 3: layer-2 aggregation + classifier --------------------
    print("build3...", flush=True)
    nc3 = _build_aggregate(K2, layer=2)
    vecs2 = np.zeros((P, 2 * C_OUT), np.float32)
    vecs2[:, 0:C_OUT] = b2[None, :]
    in3 = []
    for c in range(NCORES):
        adg = np.zeros((SHP, 1), np.float32)
        adg[:SH, 0] = ad2[pos2[c * SH + orders2[c]]]
        in3.append({"tab": tab2, "idx": idx2[c], "adg": adg,
                    "vecs": vecs2, "w2": W2})
    _t = _time.time()
    r3 = _run_retry(nc3, in3, list(range(NCORES)))
    LAST_TIMINGS["launch3"] = _time.time() - _t
    print("launch3 done", flush=True)

    out = np.zeros((N, C_OUT), np.float32)
    for c in range(NCORES):
        out[c * SH + orders2[c]] = r3.results[c]["y"][:SH, :]
    return out


# revision 13
# speedup vs baseline: 4896.1910x; 1.3257x over previous
"""GAT (2-layer, PyG-style GATConv) on 8 Trainium2 NeuronCores.

Strategy (dst-sharded, per sharding hint):
- Nodes sharded by dst across 8 cores (12500 each). Edges partitioned by dst
  core; segment-softmax + weighted aggregation happen locally per dst shard.
- Per-edge source rows (h[src], a_src[src]) are fetched with the custom SWDGE
  dma_gather instruction (68B payload rows at 256B stride), 4 table chunks of
  <=25001 rows each to fit int16 indices, round-robined over 4 SWDGE queues.
- Per dst-node tile of 128 (degree-sorted, rectangular per-chunk slot grids),
  softmax + weighted reduction run as wide DVE/ACT ops over [128, K, 17].
- 3 SPMD launches: transform (x@W1 + scores) / layer-1 aggregation + layer-2
  table build / layer-2 aggregation + classifier + log_softmax.
"""

import numpy as np

import concourse.ap_utils as ap_utils
import concourse.bacc as bacc
import concourse.bass as bass
import concourse.mybir as mybir
from concourse.bass import round_up_to_multiple
from concourse.bass_utils import run_bass_kernel_spmd
from concourse.masks import make_identity
from concourse.tile import TileContext

P = 128
NCORES = 8
N = 100000
F_IN = 512
HID = 16
C_OUT = 32
NEG_SLOPE = 0.2
W = HID + 1            # gathered row payload: h (16) + a_src (1)
ROW = 64               # table row stride in fp32 (256B, dma_gather requirement)
CHUNK = 25000          # real rows per index chunk
CHROWS = CHUNK + 1     # +1 dummy row per chunk
NCHUNK = 4
SH = N // NCORES       # real nodes per core
T_TILES = (SH + P - 1) // P
SHP = T_TILES * P      # padded shard size (12544)
NTAB = NCHUNK * CHROWS  # table rows (100004)
NEG_BIG = -1.0e30
MAX_IDX_PER_GATHER = 8192

FP = mybir.dt.float32
I16 = mybir.dt.int16


def _my_dma_gather(gp, out_ap, in_ap, idxs_ap, num_idxs, elem_size,
                   elem_step, queue_num):
    """BassGpSimd.dma_gather (non-transpose, DRAM source) without the
    256B-elem_size restriction; the row stride (elem_step) must still be a
    multiple of 256B."""
    assert idxs_ap.dtype == I16
    assert in_ap.dtype == out_ap.dtype
    assert in_ap.space == bass.MemorySpace.DRAM
    assert idxs_ap.space == bass.MemorySpace.SBUF
    assert out_ap.space == bass.MemorySpace.SBUF
    assert ap_utils.ap_is_contiguous(out_ap.ap[1:])
    assert ap_utils.ap_is_contiguous(idxs_ap.ap[1:])
    assert in_ap.ap[-1][1] == out_ap.ap[-1][1] == elem_size
    assert out_ap.ap[0][1] * out_ap.ap[1][1] == round_up_to_multiple(num_idxs, 128)
    assert in_ap.ap[0][0] == elem_step
    stride_bytes = elem_step * mybir.dt.size(in_ap.dtype)
    assert stride_bytes % 256 == 0 and stride_bytes // 256 < 256
    _in_ap = gp.lower_ap_dma(in_ap, for_custom_bir_dma=True)
    _idxs_ap = gp.lower_ap(idxs_ap)
    _out_ap = gp.lower_ap(out_ap)
    return gp.add_instruction(
        mybir.InstDMAGatherAnt(
            name=gp.bass.get_next_instruction_name(),
            ins=[*_in_ap, _idxs_ap, gp.lower_val_access(gp.to_reg(num_idxs))],
            outs=[_out_ap],
            transpose=False,
            num_idxs=num_idxs,
            elem_size=elem_size,
            stride_bytes_256=stride_bytes // 256,
            gen_mode=0,
            single_packet=False,
            queue_num=queue_num,
        )
    )


# ---------------------------------------------------------------------------
# Host-side preprocessing
# ---------------------------------------------------------------------------

def _table_row_of(pos):
    """Map a logical position 0..N-1 to its padded table row (dummies at the
    end of each chunk)."""
    return (pos // CHUNK) * CHROWS + (pos % CHUNK)


def _edge_lists(edge_index):
    """Per-core edge lists (with self loops) and degree-sorted node order."""
    src = np.asarray(edge_index[0], dtype=np.int64)
    dst = np.asarray(edge_index[1], dtype=np.int64)
    core = (dst // SH).astype(np.int32)
    lists, orders = [], []
    for c in range(NCORES):
        m = core == c
        s_c = src[m].astype(np.int64)
        d_loc = (dst[m] - c * SH).astype(np.int64)
        own = np.arange(SH, dtype=np.int64)
        s_all = np.concatenate([s_c, own + c * SH])
        d_all = np.concatenate([d_loc, own])
        deg = np.bincount(d_all, minlength=SH)
        order = np.argsort(-deg, kind="stable").astype(np.int64)
        lists.append((s_all, d_all))
        orders.append(order)
    return lists, orders


def _orders_for(lists, src_pos_map):
    """Per-core node order clustering nodes by (degree bucket, per-chunk
    counts) so per-(tile, chunk) maxima sit close to the means."""
    orders = []
    for c in range(NCORES):
        s_all, d_all = lists[c]
        pos = s_all if src_pos_map is None else src_pos_map[s_all]
        ch = pos // CHUNK
        cnt = np.zeros((SH, NCHUNK), np.int32)
        np.add.at(cnt, (d_all, ch), 1)
        mx = cnt.max(axis=1)
        orders.append(np.lexsort(
            (-cnt[:, 1], -cnt[:, 0], -mx)).astype(np.int64))
    return orders


def _schedule(lists, orders, src_pos_map):
    """Per-(tile, chunk) slot schedule shared across cores, plus per-core
    slot->source-position arrays. src_pos_map maps original src id to its
    table position (None = identity). Chunk of an edge = position // CHUNK.

    srcpos[c][slot] holds the source POSITION (in the table's logical
    numbering) or -1 for padding, slots enumerated tile-major, then chunk,
    then slot row j, then partition p.
    """
    chs, poss = [], []
    counts = []
    for c in range(NCORES):
        s_all, d_all = lists[c]
        pos = s_all if src_pos_map is None else src_pos_map[s_all]
        ch = (pos // CHUNK).astype(np.int64)
        cnt = np.zeros((SH, NCHUNK), np.int32)
        np.add.at(cnt, (d_all, ch), 1)
        counts.append(cnt)
        chs.append(ch)
        poss.append(pos)

    K = np.zeros((T_TILES, NCHUNK), np.int32)
    for c in range(NCORES):
        cnt_sorted = counts[c][orders[c]]
        pad = np.zeros((SHP - SH, NCHUNK), np.int32)
        cs = np.concatenate([cnt_sorted, pad]).reshape(T_TILES, P, NCHUNK)
        K = np.maximum(K, cs.max(axis=1))
    K = np.maximum(K, 1)
    assert int(K.max()) * P <= MAX_IDX_PER_GATHER

    slab_off = np.zeros((T_TILES, NCHUNK), np.int64)
    acc = 0
    for t in range(T_TILES):
        for cc in range(NCHUNK):
            slab_off[t, cc] = acc
            acc += int(K[t, cc]) * P

    srcpos = []
    for c in range(NCORES):
        s_all, d_all = lists[c]
        ch, pos = chs[c], poss[c]
        order = orders[c]
        gridpos_of_node = np.full(SH, -1, np.int64)
        gridpos_of_node[order] = np.arange(SH)
        gp_e = gridpos_of_node[d_all]
        t_e = gp_e // P
        p_e = gp_e % P
        bucket = gp_e * NCHUNK + ch
        bo = np.argsort(bucket, kind="stable")
        bsort = bucket[bo]
        rank = np.arange(len(bsort)) - np.searchsorted(bsort, bsort, side="left")
        rank_e = np.empty_like(rank)
        rank_e[bo] = rank
        total = int(K.sum()) * P
        arr = np.full(total, -1, np.int64)
        slotpos = slab_off[t_e, ch] + rank_e * P + p_e
        arr[slotpos] = pos
        srcpos.append(arr)
    return K, srcpos


def _wrap_idx(local_idx):
    """Wrap an int16 index list [M] (M % 128 == 0) into the SWDGE layout
    [128, M//16]: idx i at partition i%16, col i//16, replicated x8."""
    M = local_idx.shape[0]
    w = local_idx.reshape(M // 16, 16).T.astype(np.int16)  # [16, M//16]
    return np.tile(w, (8, 1))


def _build_idx_tensor(srcpos_arr):
    """Translate slot source positions to chunk-local padded-table indices and
    wrap. Padding slots (-1) use the local dummy index CHUNK. The chunk of a
    slot is fixed by the slab structure, so the local index is pos % CHUNK."""
    a = srcpos_arr
    out = np.empty(a.shape[0], np.int16)
    pad = a < 0
    out[~pad] = (a[~pad] % CHUNK).astype(np.int16)
    out[pad] = CHUNK
    return _wrap_idx(out)


# ---------------------------------------------------------------------------
# Device programs
# ---------------------------------------------------------------------------

def _build_transform():
    """Launch 1: per core, h = xT_shard.T @ W1, a_s = h@att_src, a_d = h@att_dst.
    Inputs : xt [F_IN, SH] fp32 (pre-transposed shard), w1 [F_IN//P, P, HID],
             att [128, 2*HID] (att_src tiled | att_dst tiled)
    Outputs: tabs [SHP, ROW] (cols 0:17 = h|a_s), ad [SHP, 1]
    """
    nc = bacc.Bacc("TRN2", target_bir_lowering=False, debug=False,
                   num_devices=NCORES)
    xt = nc.dram_tensor("xt", [F_IN, SH], FP, kind="ExternalInput").ap()
    w1 = nc.dram_tensor("w1", [F_IN // P, P, HID], FP, kind="ExternalInput").ap()
    att = nc.dram_tensor("att", [P, 2 * HID], FP, kind="ExternalInput").ap()
    tabs = nc.dram_tensor("tabs", [SHP, ROW], FP, kind="ExternalOutput").ap()
    ad = nc.dram_tensor("ad", [SHP, 1], FP, kind="ExternalOutput").ap()
    KC = F_IN // P
    with TileContext(nc) as tc:
        with tc.tile_pool(name="cst", bufs=1) as cst, \
             tc.tile_pool(name="xk", bufs=3) as xk, \
             tc.tile_pool(name="hp", bufs=3) as hp, \
             tc.tile_pool(name="ps", bufs=2, space="PSUM") as ps:
            w1t = cst.tile([P, KC * HID], FP)
            nc.sync.dma_start(out=w1t[:].rearrange("p (k h) -> p k h", k=KC),
                              in_=w1[:].rearrange("k p h -> p k h"))
            attt = cst.tile([P, 2 * HID], FP)
            nc.sync.dma_start(out=attt[:], in_=att[:])
            for t in range(T_TILES):
                m0 = t * P
                mn = min(P, SH - m0)
                xtile = xk.tile([P, KC * P], FP)
                nc.sync.dma_start(
                    out=xtile[:].rearrange("p (k m) -> p k m", k=KC)[:, :, 0:mn],
                    in_=xt[:, m0:m0 + mn].rearrange("(k p) m -> p k m", p=P))
                psum = ps.tile([P, HID], FP, space="PSUM")
                for k in range(KC):
                    nc.tensor.matmul(
                        psum[:mn, :],
                        lhsT=xtile[:, k * P:k * P + mn],
                        rhs=w1t[:, k * HID:(k + 1) * HID],
                        start=(k == 0), stop=(k == KC - 1))
                row = hp.tile([P, W], FP)
                adcol = hp.tile([P, 1], FP)
                if mn < P:
                    nc.vector.memset(row[:], 0.0)
                nc.scalar.copy(row[:mn, 0:HID], psum[:mn, 0:HID])
                scr1 = hp.tile([P, HID], FP, tag="scratch")
                nc.vector.tensor_tensor(out=scr1[:], in0=row[:, 0:HID],
                                        in1=attt[:, 0:HID],
                                        op=mybir.AluOpType.mult)
                nc.vector.tensor_reduce(row[:, HID:HID + 1], scr1[:],
                                        axis=mybir.AxisListType.X,
                                        op=mybir.AluOpType.add)
                scr2 = hp.tile([P, HID], FP, tag="scratch2")
                nc.vector.tensor_tensor(out=scr2[:], in0=row[:, 0:HID],
                                        in1=attt[:, HID:2 * HID],
                                        op=mybir.AluOpType.mult)
                nc.vector.tensor_reduce(adcol[:], scr2[:],
                                        axis=mybir.AxisListType.X,
                                        op=mybir.AluOpType.add)
                nc.sync.dma_start(out=tabs[m0:m0 + P, 0:W], in_=row[:])
                nc.sync.dma_start(out=ad[m0:m0 + P, :], in_=adcol[:])
    nc.compile()
    return nc


def _build_aggregate(K, layer):
    """Launches 2 & 3: grid gather + segment softmax + weighted aggregation.

    layer == 1:
      out per tile: h' = relu(num/den + b1); table2 row [h'|a_s2]; ad2.
      Inputs: tab [NTAB, ROW], idx [128, TOTW], adg [SHP, 1],
              vecs [128, 4*HID] = (b1 | u2 | v2 | unused) tiled
      Outputs: tabs [SHP, ROW], ad [SHP, 1]
    layer == 2:
      out per tile: log_softmax(num/den @ W2 + b2)
      Inputs: tab, idx, adg, vecs [128, 2*C_OUT] = (b2 | unused), w2 [HID, C_OUT]
      Outputs: y [SHP, C_OUT]
    """
    nc = bacc.Bacc("TRN2", target_bir_lowering=False, debug=False,
                   num_devices=NCORES, num_swdge_queues=4)
    tot_slots = int(K.sum()) * P
    TOTW = tot_slots // 16
    tab = nc.dram_tensor("tab", [NTAB, ROW], FP, kind="ExternalInput").ap()
    idx = nc.dram_tensor("idx", [P, TOTW], I16, kind="ExternalInput").ap()
    adg = nc.dram_tensor("adg", [SHP, 1], FP, kind="ExternalInput").ap()
    if layer == 1:
        vecs = nc.dram_tensor("vecs", [P, 4 * HID], FP, kind="ExternalInput").ap()
        tabs = nc.dram_tensor("tabs", [SHP, ROW], FP, kind="ExternalOutput").ap()
        ad = nc.dram_tensor("ad", [SHP, 1], FP, kind="ExternalOutput").ap()
    else:
        vecs = nc.dram_tensor("vecs", [P, 2 * C_OUT], FP, kind="ExternalInput").ap()
        w2 = nc.dram_tensor("w2", [HID, C_OUT], FP, kind="ExternalInput").ap()
        y = nc.dram_tensor("y", [SHP, C_OUT], FP, kind="ExternalOutput").ap()

    Ktot = K.sum(axis=1)  # slots per node per tile
    qn = [0]

    with TileContext(nc) as tc:
        with tc.tile_pool(name="cst", bufs=1) as cst, \
             tc.tile_pool(name="ix", bufs=3) as ixp, \
             tc.tile_pool(name="gr", bufs=3) as grp, \
             tc.tile_pool(name="sc", bufs=3) as scp, \
             tc.tile_pool(name="ou", bufs=3) as oup, \
             tc.tile_pool(name="ps", bufs=2, space="PSUM") as ps:
            vt = cst.tile([P, vecs.shape[1]], FP)
            nc.sync.dma_start(out=vt[:], in_=vecs[:])
            if layer == 2:
                w2t = cst.tile([HID, C_OUT], FP)
                nc.sync.dma_start(out=w2t[:], in_=w2[:])
                ident = cst.tile([P, P], FP)
                make_identity(nc, ident[:])
            slot_off = 0   # running slot offset (per-partition slots)
            for t in range(T_TILES):
                kt = int(Ktot[t])
                g = grp.tile([P, kt * W], FP, tag="grid")
                idx_t = ixp.tile([P, kt * 8], I16, tag="idx")
                nc.sync.dma_start(
                    out=idx_t[:],
                    in_=idx[:, slot_off * 8:(slot_off + kt) * 8])
                coff = 0
                for cc in range(NCHUNK):
                    kc = int(K[t, cc])
                    ni = kc * P
                    assert ni <= MAX_IDX_PER_GATHER
                    _my_dma_gather(
                        nc.gpsimd,
                        g[:, coff * W:(coff + kc) * W].rearrange(
                            "p (k w) -> p k w", w=W),
                        tab[cc * CHROWS:, 0:W],
                        idx_t[:, coff * 8:(coff + kc) * 8],
                        ni, W, ROW, qn[0] % 4)
                    qn[0] += 1
                    coff += kc
                adcol = scp.tile([P, 1], FP, tag="adc")
                nc.sync.dma_start(out=adcol[:], in_=adg[t * P:(t + 1) * P, :])
                # e = leaky_relu(a_s + a_d) over [P, kt]
                e = scp.tile([P, kt], FP, tag="e")
                pre = scp.tile([P, kt], FP, tag="pre")
                neg = scp.tile([P, kt], FP, tag="neg")
                as_view = g[:].rearrange("p (k w) -> p k w", w=W)[:, :, HID:HID + 1]
                nc.vector.tensor_scalar_add(
                    pre[:], as_view.rearrange("p k w -> p (k w)"), adcol[:])
                nc.vector.tensor_scalar_min(neg[:], pre[:], 0.0)
                nc.vector.tensor_scalar_max(e[:], pre[:], 0.0)
                nc.vector.tensor_scalar(
                    out=neg[:], in0=neg[:], scalar1=NEG_SLOPE, scalar2=None,
                    op0=mybir.AluOpType.mult)
                nc.vector.tensor_tensor(out=e[:], in0=e[:], in1=neg[:],
                                        op=mybir.AluOpType.add)
                m = scp.tile([P, 1], FP, tag="m")
                nc.vector.tensor_reduce(m[:], e[:], axis=mybir.AxisListType.X,
                                        op=mybir.AluOpType.max, negate=True)
                # m now holds -max; w = exp(e - max), den = sum w
                wts = scp.tile([P, kt], FP, tag="w")
                den = scp.tile([P, 1], FP, tag="den")
                nc.scalar.activation(
                    wts[:], e[:], mybir.ActivationFunctionType.Exp,
                    bias=m[:], scale=1.0, accum_out=den[:])
                inv = scp.tile([P, 1], FP, tag="inv")
                nc.vector.reciprocal(inv[:], den[:])
                nc.vector.tensor_scalar_mul(wts[:], wts[:], inv[:])
                # g *= alpha (broadcast over W columns)
                nc.vector.tensor_tensor(
                    out=g[:].rearrange("p (k w) -> p k w", w=W),
                    in0=g[:].rearrange("p (k w) -> p k w", w=W),
                    in1=wts[:].to_broadcast([P, kt, W]),
                    op=mybir.AluOpType.mult)
                num = oup.tile([P, W], FP, tag="num")
                gv = g[:].rearrange("p (k w) -> p w k", w=W)
                nc.vector.tensor_reduce(num[:], gv, axis=mybir.AxisListType.X,
                                        op=mybir.AluOpType.add)
                if layer == 1:
                    row = oup.tile([P, W], FP, tag="row")
                    adout = oup.tile([P, 1], FP, tag="ado")
                    # h' = relu(num + b1)
                    nc.vector.tensor_tensor(
                        out=row[:, 0:HID], in0=num[:, 0:HID],
                        in1=vt[:, 0:HID], op=mybir.AluOpType.add)
                    nc.vector.tensor_scalar_max(row[:, 0:HID], row[:, 0:HID], 0.0)
                    scr1 = oup.tile([P, HID], FP, tag="s1")
                    nc.vector.tensor_tensor(out=scr1[:], in0=row[:, 0:HID],
                                            in1=vt[:, HID:2 * HID],
                                            op=mybir.AluOpType.mult)
                    nc.vector.tensor_reduce(row[:, HID:HID + 1], scr1[:],
                                            axis=mybir.AxisListType.X,
                                            op=mybir.AluOpType.add)
                    scr2 = oup.tile([P, HID], FP, tag="s2")
                    nc.vector.tensor_tensor(out=scr2[:], in0=row[:, 0:HID],
                                            in1=vt[:, 2 * HID:3 * HID],
                                            op=mybir.AluOpType.mult)
                    nc.vector.tensor_reduce(adout[:], scr2[:],
                                            axis=mybir.AxisListType.X,
                                            op=mybir.AluOpType.add)
                    nc.sync.dma_start(out=tabs[t * P:(t + 1) * P, 0:W], in_=row[:])
                    nc.sync.dma_start(out=ad[t * P:(t + 1) * P, :], in_=adout[:])
                else:
                    # out2 = num[:, :16] @ W2 + b2 -> log_softmax
                    pT = ps.tile([HID, P], FP, space="PSUM", tag="pT")
                    nc.tensor.transpose(pT[:], num[:, 0:HID], ident[:])
                    nT = oup.tile([HID, P], FP, tag="nT")
                    nc.scalar.copy(nT[:], pT[:])
                    p2 = ps.tile([P, C_OUT], FP, space="PSUM", tag="p2")
                    nc.tensor.matmul(p2[:], lhsT=nT[:], rhs=w2t[:],
                                     start=True, stop=True)
                    o = oup.tile([P, C_OUT], FP, tag="o")
                    nc.vector.tensor_tensor(out=o[:], in0=p2[:],
                                            in1=vt[:, 0:C_OUT],
                                            op=mybir.AluOpType.add)
                    mx = scp.tile([P, 1], FP, tag="mx")
                    nc.vector.tensor_reduce(mx[:], o[:],
                                            axis=mybir.AxisListType.X,
                                            op=mybir.AluOpType.max, negate=True)
                    ex = oup.tile([P, C_OUT], FP, tag="ex")
                    se = scp.tile([P, 1], FP, tag="se")
                    nc.scalar.activation(ex[:], o[:],
                                         mybir.ActivationFunctionType.Exp,
                                         bias=mx[:], scale=1.0, accum_out=se[:])
                    ls = scp.tile([P, 1], FP, tag="ls")
                    nc.scalar.activation(ls[:], se[:],
                                         mybir.ActivationFunctionType.Ln)
                    ofs = scp.tile([P, 1], FP, tag="ofs")
                    # ofs = mx(-max) - ln(se);  out = o + ofs
                    nc.vector.tensor_tensor(out=ofs[:], in0=mx[:],
                                            in1=ls[:],
                                            op=mybir.AluOpType.subtract)
                    nc.scalar.activation(o[:], o[:],
                                         mybir.ActivationFunctionType.Identity,
                                         bias=ofs[:], scale=1.0)
                    nc.sync.dma_start(out=y[t * P:(t + 1) * P, :], in_=o[:])
                slot_off += kt
    nc.compile()
    return nc


# ---------------------------------------------------------------------------
# Main entry
# ---------------------------------------------------------------------------

LAST_TIMINGS = {}
LAST_STATS = {}


def _run_retry(nc, in_maps, cores):
    try:
        return run_bass_kernel_spmd(nc, in_maps, cores)
    except Exception:
        # transient accelerator-unrecoverable states heal on retry
        return run_bass_kernel_spmd(nc, in_maps, cores)


def kernel(x, edge_index, W1, att_src1, att_dst1, b1, W2, att_src2, att_dst2, b2):
    import time as _time
    x = np.asarray(x, np.float32)
    W1 = np.asarray(W1, np.float32)
    W2 = np.asarray(W2, np.float32)
    att_src1 = np.asarray(att_src1, np.float32)
    att_dst1 = np.asarray(att_dst1, np.float32)
    att_src2 = np.asarray(att_src2, np.float32)
    att_dst2 = np.asarray(att_dst2, np.float32)
    b1 = np.asarray(b1, np.float32)
    b2 = np.asarray(b2, np.float32)

    print("preprocess...", flush=True)
    lists, _ = _edge_lists(edge_index)
    orders1 = _orders_for(lists, None)
    # layer-2 position map: original id -> position in concat-of-sorted-shards
    pos2 = np.empty(N, np.int64)
    for c in range(NCORES):
        pos2[c * SH + orders1[c]] = c * SH + np.arange(SH)
    orders2 = _orders_for(lists, pos2)

    K1, srcpos1 = _schedule(lists, orders1, None)
    K2, srcpos2 = _schedule(lists, orders2, pos2)
    LAST_STATS["slots1"] = int(K1.sum()) * P
    LAST_STATS["slots2"] = int(K2.sum()) * P
    idx1 = [_build_idx_tensor(srcpos1[c]) for c in range(NCORES)]
    idx2 = [_build_idx_tensor(srcpos2[c]) for c in range(NCORES)]

    # ---- launch 1: transform -------------------------------------------
    print("build1...", flush=True)
    nc1 = _build_transform()
    xT = np.ascontiguousarray(x.T)
    att_t = np.tile(np.concatenate([att_src1, att_dst1])[None, :], (P, 1))
    w1r = np.ascontiguousarray(W1.reshape(F_IN // P, P, HID))
    in1 = [{"xt": np.ascontiguousarray(xT[:, c * SH:(c + 1) * SH]),
            "w1": w1r, "att": att_t.astype(np.float32)}
           for c in range(NCORES)]
    _t = _time.time()
    r1 = _run_retry(nc1, in1, list(range(NCORES)))
    LAST_TIMINGS["launch1"] = _time.time() - _t
    print("launch1 done", flush=True)
    tab_rows = np.zeros((NTAB, ROW), np.float32)
    ad1 = np.zeros(N, np.float32)
    for c in range(NCORES):
        hs = r1.results[c]["tabs"][:SH, 0:W]
        ids = c * SH + np.arange(SH)
        tab_rows[_table_row_of(ids), 0:W] = hs
        ad1[ids] = r1.results[c]["ad"][:SH, 0]
    for cc in range(NCHUNK):
        tab_rows[cc * CHROWS + CHUNK, HID] = NEG_BIG  # dummy a_src

    # ---- launch 2: layer-1 aggregation + layer-2 table -----------------
    print("build2...", flush=True)
    nc2 = _build_aggregate(K1, layer=1)
    u2 = W2 @ att_src2
    v2 = W2 @ att_dst2
    vecs1 = np.zeros((P, 4 * HID), np.float32)
    vecs1[:, 0:HID] = b1[None, :]
    vecs1[:, HID:2 * HID] = u2[None, :]
    vecs1[:, 2 * HID:3 * HID] = v2[None, :]
    in2 = []
    for c in range(NCORES):
        adg = np.zeros((SHP, 1), np.float32)
        adg[:SH, 0] = ad1[c * SH + orders1[c]]
        in2.append({"tab": tab_rows, "idx": idx1[c], "adg": adg,
                    "vecs": vecs1})
    _t = _time.time()
    r2 = _run_retry(nc2, in2, list(range(NCORES)))
    LAST_TIMINGS["launch2"] = _time.time() - _t
    print("launch2 done", flush=True)
    tab2 = np.zeros((NTAB, ROW), np.float32)
    ad2 = np.zeros(N, np.float32)
    for c in range(NCORES):
        hs = r2.results[c]["tabs"][:SH, 0:W]
        posn = c * SH + np.arange(SH)
        tab2[_table_row_of(posn), 0:W] = hs
        ad2[posn] = r2.results[c]["ad"][:SH, 0]
    for cc in range(NCHUNK):
        tab2[cc * CHROWS + CHUNK, HID] = NEG_BIG

    # ---- launch 3: layer-2 aggregation + classifier --------------------
    print("build3...", flush=True)
    nc3 = _build_aggregate(K2, layer=2)
    vecs2 = np.zeros((P, 2 * C_OUT), np.float32)
    vecs2[:, 0:C_OUT] = b2[None, :]
    in3 = []
    for c in range(NCORES):
        adg = np.zeros((SHP, 1), np.float32)
        adg[:SH, 0] = ad2[pos2[c * SH + orders2[c]]]
        in3.append({"tab": tab2, "idx": idx2[c], "adg": adg,
                    "vecs": vecs2, "w2": W2})
    _t = _time.time()
    r3 = _run_retry(nc3, in3, list(range(NCORES)))
    LAST_TIMINGS["launch3"] = _time.time() - _t
    print("launch3 done", flush=True)

    out = np.zeros((N, C_OUT), np.float32)
    for c in range(NCORES):
        out[c * SH + orders2[c]] = r3.results[c]["y"][:SH, :]
    return out
